# revision 12
# baseline (speedup 1.0000x reference)
"""GAT layer (nn_GAT_layer_67619965108552) as a Trainium2 Bass/Tile SPMD kernel.

Same collapsed math as the previous version (see prep_check.py for the numpy
emulation), restructured for speed:

  * The separate U-pass is gone: the score vectors ride as 6 extra rhs columns
    ([wam_hi | wam_lo] fp8 hi/lo split for precision) on the out-pass matmuls,
    so one pass over xT produces out AND [u, s1, s2] per node.
  * Even/odd pair extraction (tt = s1[even]+s2[odd], u pairs for the alphas)
    happens on the PE with a single resident 0/1 stationary E_eo: one N=24
    matmul per 8 blocks into one PSUM bank, whose columns line up so every
    downstream consumer is a uniform stride-6 2D slice. No DRAM roundtrip.
  * All DRAM operands are host-pre-tiled so every DMA descriptor is a 4-8 KB
    contiguous per-partition run (the old layouts were descriptor-rate bound
    at ~1 KB/descriptor).
  * Stage B runs in mb-slab order: adj arrives as 8 slabs of [all j, 128 i],
    each slab's 64 matmuls accumulate Y1/Y2 for one output row-block, and the
    per-block epilogue (alpha combine, normalize, sigmoid, store) overlaps the
    next slab's matmuls. No epilogue tail.
  * Stage B is all-fp8 (adj 0/1 exact; fo = f*out pre-scaled by 1/4 so the
    normalizer ratio is unchanged and values stay far below the TRN fp8e4
    240 cap). Numpy-emulated end-to-end rel err: 2.5e-3.

Sharding: rows of adj (and the output) across 8 cores, 1024 rows each;
x/weight replicated; every core computes the full out = x@W.
"""
import numpy as np

import concourse.bass as bass
import concourse.tile as tile
from concourse import bacc, mybir
from concourse.bass_utils import run_bass_kernel_spmd

F32 = mybir.dt.float32
F32R = mybir.dt.float32r
BF16 = mybir.dt.bfloat16
F8 = mybir.dt.float8e4

N = 8192
FIN = 512
FOUT = 256
P = 128
NB = N // P        # 64 node blocks
NCORES = 8
RPC = N // NCORES  # 1024 output rows per core
MB = RPC // P      # 8 output row blocks per core
NS = 8             # xT slices
SC = 0.25          # fo pre-scale (cancels in the normalizer ratio)


def build_program():
    nc = bacc.Bacc("TRN2", target_bir_lowering=False, debug=False,
                   num_devices=NCORES)

    xt_d = nc.dram_tensor("xt", [NS, P, 4 * 1024], F8, kind="ExternalInput")
    wrhs_d = nc.dram_tensor("wrhs", [P, 4 * 262], F8, kind="ExternalInput")
    geo_d = nc.dram_tensor("geo", [P, 4 * P], F32R, kind="ExternalInput")
    adjt_d = nc.dram_tensor("adjt", [MB, P, NB * P], F8, kind="ExternalInput")
    # gcol cols: [g*SC, (1-g)*SC, 0, 1-g]
    g_d = nc.dram_tensor("gcol", [P, 4], F32, kind="ExternalInput")
    # selg[p, bp, B] = g * (B == 8c + bp)
    selg_d = nc.dram_tensor("selg", [P, MB, 32], F32, kind="ExternalInput")
    y_d = nc.dram_tensor("y", [RPC, FOUT], F32, kind="ExternalOutput")

    with tile.TileContext(nc) as tc:
        with tc.tile_pool(name="const", bufs=1) as constp, \
             tc.tile_pool(name="adjp", bufs=3) as adjp, \
             tc.tile_pool(name="sa", bufs=3) as sa:

            # ---- constants (scalar HWDGE ring; sync ring streams xT) ----
            wrhs = constp.tile([P, 4, 262], F8)
            nc.scalar.dma_start(wrhs[:], wrhs_d.ap().rearrange(
                "p (c v) -> p c v", v=262))
            geo = constp.tile([P, 4, P], F32R)
            nc.scalar.dma_start(geo[:], geo_d.ap().rearrange(
                "p (g q) -> p g q", q=P))
            gcol = constp.tile([P, 4], F32)
            nc.scalar.dma_start(gcol[:], g_d.ap())
            selg = constp.tile([P, MB, 32], F32)
            nc.scalar.dma_start(selg[:], selg_d.ap())

            dumm = constp.tile([1, 2], F32)
            nc.gpsimd.memset(dumm[:], 0.0)

            # persistent staging: out blocks (col 256 preset to 1.0 so the
            # f-scale pass emits the normalizer column for free)
            outb = [constp.tile([P, FOUT + 1], BF16, name=f"outb{b}")
                    for b in range(NB)]
            for b in range(NB):
                nc.gpsimd.memset(outb[b][:, FOUT:FOUT + 1], 1.0)
            fo8 = [constp.tile([P, FOUT + 1], F8, name=f"fo8_{b}")
                   for b in range(NB)]
            uc6 = constp.tile([P, NB, 6], F32)
            ucols = constp.tile([P, NB, 3], F32R)
            fpm = constp.tile([P, 32], F32)
            al1 = constp.tile([P, MB], F32)
            al2 = constp.tile([P, MB], F32)

            # ---- fused out+score pass over the xT stream ----
            with tc.tile_pool(name="xtp", bufs=NS) as xtp, \
                 tc.tile_pool(name="ps_o", bufs=4, space="PSUM") as ps_o, \
                 tc.tile_pool(name="ps_e", bufs=1, space="PSUM") as ps_e:

                # eoX cols: [tts 32 | ae 32 | be 32], partition-aligned
                eoX = ps_e.tile([P, 96], F32, name="eoX")

                def extract_half(lo, hi):
                    # rhs: even/odd blocks for kc in [lo, hi)
                    ev = ucols[:, 2 * lo:2 * hi:2, :]
                    od = ucols[:, 2 * lo + 1:2 * hi:2, :]
                    for base, pairs in ((0, ((0, ev, 1), (1, ev, 2),
                                             (2, od, 1), (3, od, 2))),
                                        (32, ((0, ev, 0), (2, od, 0))),
                                        (64, ((1, ev, 0), (3, od, 0)))):
                        dst = eoX[:, base + lo:base + hi]
                        for k, (g, rr, v) in enumerate(pairs):
                            nc.tensor.matmul(dst, geo[:, g, :], rr[:, :, v],
                                             start=(k == 0),
                                             stop=(k == len(pairs) - 1))

                def fpm_half(lo, hi):
                    hs = slice(lo, hi)
                    lrt = sa.tile([P, hi - lo], F32, tag="lrt", name="lrt")
                    nc.vector.tensor_scalar_mul(lrt[:], eoX[:, hs], 0.01)
                    nc.vector.tensor_max(lrt[:], eoX[:, hs], lrt[:])
                    ext = sa.tile([P, hi - lo], F32, tag="ext", name="ext")
                    nc.scalar.activation(ext[:], lrt[:],
                                         mybir.ActivationFunctionType.Exp)
                    nc.vector.tensor_scalar(fpm[:, hs], ext[:], gcol[:, 1:2],
                                            gcol[:, 0:1],
                                            op0=mybir.AluOpType.mult,
                                            op1=mybir.AluOpType.add)

                def scale_one(jc):
                    kc = jc % 32
                    if jc % 2 == 0:
                        nc.scalar.activation(
                            fo8[jc][:], outb[jc][:],
                            mybir.ActivationFunctionType.Copy,
                            scale=fpm[:, kc:kc + 1])
                    else:
                        nc.vector.tensor_scalar_mul(fo8[jc][:], outb[jc][:],
                                                    fpm[:, kc:kc + 1])

                def half_jcs(h):
                    return list(range(16 * h, 16 * h + 16)) + \
                        list(range(32 + 16 * h, 32 + 16 * h + 16))

                pend_scale = []
                for s in range(NS):
                    if s == 5:
                        pend_scale = [jc for jc in half_jcs(0)
                                      if not 40 <= jc <= 47]
                    xs = xtp.tile([P, 4096], F8, tag="xts", name=f"xt{s}")
                    nc.sync.dma_start(xs[:], xt_d.ap()[s])
                    for b8 in range(8):
                        b = s * 8 + b8
                        po = ps_o.tile([P, 262], F32, tag="po", name="po")
                        for c in range(4):
                            nc.tensor.matmul(
                                po[:],
                                xs[:, c * 1024 + b8 * P:
                                   c * 1024 + (b8 + 1) * P],
                                wrhs[:, c, :], start=(c == 0), stop=(c == 3))
                        if b == 1:
                            # exp-table warm, anchored mid-stream
                            nc.scalar.activation(
                                dumm[:, 0:1], ucols[0:1, 0, 0:1],
                                mybir.ActivationFunctionType.Exp)
                        if b == 34:
                            # first-quarter extraction rides mid-stream
                            # (deps on slices 0-3 are settled by now)
                            extract_half(0, 16)
                            fpm_half(0, 16)
                        if b == 50:
                            extract_half(16, 24)
                            fpm_half(16, 24)
                        if b == 52:
                            pend_scale += [jc for jc in range(16, 24)] + \
                                [jc for jc in range(48, 56)]
                        if b >= 36 and pend_scale:
                            # drip fo8 scales through the copy stream so the
                            # FIFO ACT/DVE queues never stall the PE
                            scale_one(pend_scale.pop(0))
                        if 40 <= b <= 47:
                            # kc = b-32 is in fpm half 0 (ready by now):
                            # fuse the f-scale into the PSUM drain, skip
                            # outb staging and the separate scale op
                            kc = b - 32
                            if b % 2 == 0:
                                nc.scalar.activation(
                                    fo8[b][:, 0:FOUT], po[:, 0:FOUT],
                                    mybir.ActivationFunctionType.Copy,
                                    scale=fpm[:, kc:kc + 1])
                                nc.vector.tensor_copy(uc6[:, b, :],
                                                      po[:, 256:262])
                                nc.vector.tensor_copy(fo8[b][:, FOUT:],
                                                      fpm[:, kc:kc + 1])
                            else:
                                nc.vector.tensor_scalar_mul(
                                    fo8[b][:, 0:FOUT], po[:, 0:FOUT],
                                    fpm[:, kc:kc + 1])
                                nc.scalar.copy(uc6[:, b, :], po[:, 256:262])
                                nc.scalar.copy(fo8[b][:, FOUT:],
                                               fpm[:, kc:kc + 1])
                        elif b % 2 == 0:
                            nc.scalar.copy(outb[b][:, 0:FOUT], po[:, 0:FOUT])
                            nc.vector.tensor_copy(uc6[:, b, :],
                                                  po[:, 256:262])
                        else:
                            nc.vector.tensor_copy(outb[b][:, 0:FOUT],
                                                  po[:, 0:FOUT])
                            nc.scalar.copy(uc6[:, b, :], po[:, 256:262])
                    sl = slice(s * 8, (s + 1) * 8)
                    nc.vector.tensor_add(ucols[:, sl, :], uc6[:, sl, 0:3],
                                         uc6[:, sl, 3:6])
                extract_half(24, 32)
                fpm_half(24, 32)
                for jc in pend_scale + list(range(24, 32)) + \
                        list(range(56, 64)):
                    scale_one(jc)
                # sigmoid-table warm, anchored on fpm
                nc.scalar.activation(dumm[:, 1:2], fpm[0:1, 0:1],
                                     mybir.ActivationFunctionType.Sigmoid)

                # alphas: aev/bev = exp(lrelu(ae/be)), al = selg-reduce + 1-g
                lra = sa.tile([P, 64], F32, tag="lra", name="lra")
                aeb = sa.tile([P, 64], F32, tag="aeb", name="aeb")
                nc.vector.tensor_copy(aeb[:, 0:32], eoX[:, 32:64])
                nc.vector.tensor_copy(aeb[:, 32:64], eoX[:, 64:96])
                nc.vector.tensor_scalar_mul(lra[:], aeb[:], 0.01)
                nc.vector.tensor_max(lra[:], aeb[:], lra[:])
                nc.scalar.activation(aeb[:], lra[:],
                                     mybir.ActivationFunctionType.Exp)
                for bp in range(MB):
                    m1 = sa.tile([P, 32], F32, tag="alm", name="alm1")
                    nc.vector.tensor_mul(m1[:], aeb[:, 0:32], selg[:, bp, :])
                    nc.vector.tensor_reduce(al1[:, bp:bp + 1], m1[:],
                                            axis=mybir.AxisListType.X,
                                            op=mybir.AluOpType.add)
                    m2 = sa.tile([P, 32], F32, tag="alm", name="alm2")
                    nc.vector.tensor_mul(m2[:], aeb[:, 32:64], selg[:, bp, :])
                    nc.vector.tensor_reduce(al2[:, bp:bp + 1], m2[:],
                                            axis=mybir.AxisListType.X,
                                            op=mybir.AluOpType.add)
                nc.vector.tensor_scalar_add(al1[:], al1[:], gcol[:, 3:4])
                nc.vector.tensor_scalar_add(al2[:], al2[:], gcol[:, 3:4])

            # ---- f-scale pass (jc order = stage-B consumption order) ----
            with tc.tile_pool(name="ps_y", bufs=4, space="PSUM") as ps_y, \
                 tc.tile_pool(name="comb", bufs=3) as comb:

                # ---- adj slabs (SWDGE; early ones gated on out-pass
                # progress so they don't steal HBM from the xT stream) ----
                def slab_load(mb, marker=None):
                    t = adjp.tile([P, NB * P], F8, tag="slab",
                                  name=f"slab{mb}")
                    if marker is not None:
                        nc.scalar.copy(t[0:1, 0:1], marker)
                    nc.gpsimd.dma_start(t[:], adjt_d.ap()[mb])
                    return t

                slabs = [slab_load(0, ucols[0:1, 8, 0:1]),
                         slab_load(1, ucols[0:1, 24, 0:1]),
                         slab_load(2, ucols[0:1, 40, 0:1])]
                slabs += [slab_load(mb) for mb in range(3, MB)]

                # ---- stage B: per row-block accumulate + inline epilogue ----
                for mb in range(MB):
                    at = slabs[mb]
                    ya = ps_y.tile([P, FOUT + 1], F32, tag="ya", name="ya")
                    yb = ps_y.tile([P, FOUT + 1], F32, tag="yb", name="yb")
                    for jc in range(NB):
                        dst = ya if jc < 32 else yb
                        nc.tensor.matmul(dst[:], at[:, jc * P:(jc + 1) * P], fo8[jc][:],
                                         start=(jc % 32 == 0),
                                         stop=(jc % 32 == 31))
                    za = comb.tile([P, FOUT + 1], F32, tag="za", name="za")
                    nc.scalar.activation(za[:], ya[:],
                                         mybir.ActivationFunctionType.Copy,
                                         scale=al1[:, mb:mb + 1])
                    t2 = comb.tile([P, FOUT + 1], F32, tag="t2", name="t2")
                    nc.vector.tensor_scalar_mul(t2[:], yb[:],
                                                al2[:, mb:mb + 1])
                    z2 = comb.tile([P, FOUT + 1], F32, tag="z2", name="z2")
                    nc.vector.tensor_add(z2[:], za[:], t2[:])
                    rec = comb.tile([P, 1], F32, tag="rec", name="rec")
                    nc.vector.reciprocal(rec[:], z2[:, FOUT:FOUT + 1])
                    res = comb.tile([P, FOUT], F32, tag="res", name="res")
                    resg = comb.tile([P, FOUT], F32, tag="resg", name="resg")
                    for hh in range(2):
                        cs = slice(hh * 128, hh * 128 + 128)
                        nc.vector.tensor_scalar_mul(res[:, cs], z2[:, cs],
                                                    rec[:])
                        nc.scalar.activation(
                            resg[:, cs], res[:, cs],
                            mybir.ActivationFunctionType.Sigmoid)
                        nc.sync.dma_start(
                            y_d.ap()[mb * P:(mb + 1) * P, cs], resg[:, cs])

    nc.compile()
    return nc


_NC_CACHE = None


def _get_program():
    global _NC_CACHE
    if _NC_CACHE is None:
        _NC_CACHE = build_program()
    return _NC_CACHE


def make_in_maps(x, weight, att_vec, adj):
    import ml_dtypes
    f8 = ml_dtypes.float8_e4m3
    x = np.asarray(x, dtype=np.float32)
    weight = np.ascontiguousarray(np.asarray(weight, dtype=np.float32))
    att_vec = np.asarray(att_vec, dtype=np.float32)

    # xt[s, p, c*1024+n] = x[s*1024+n, c*128+p]
    x8 = x.astype(f8)
    xt = np.ascontiguousarray(
        x8.reshape(NS, 1024, 4, P).transpose(0, 3, 2, 1)).reshape(NS, P, 4096)

    a1 = att_vec[:FOUT, 0]
    a2 = att_vec[FOUT:, 0]
    att3 = np.stack([a1 + a2, a1, a2], axis=1).astype(np.float32)
    wam = (weight @ att3).astype(np.float32)            # [FIN, 3]
    wam_hi = wam.astype(f8)
    wam_lo = (wam - wam_hi.astype(np.float32)).astype(f8)
    wcat = np.concatenate([weight.astype(f8),
                           wam_hi, wam_lo], axis=1)     # [FIN, 262]
    wrhs = np.ascontiguousarray(
        wcat.reshape(4, P, 262).transpose(1, 0, 2)).reshape(P, 4 * 262)

    geo = np.zeros((4, P, P), np.float32)   # [g, p, q]
    q = np.arange(64)
    geo[0, 2 * q, q] = 1.0          # even -> top
    geo[1, 2 * q + 1, q] = 1.0      # odd  -> top
    geo[2, 2 * q, q + 64] = 1.0     # even -> bot
    geo[3, 2 * q + 1, q + 64] = 1.0  # odd -> bot
    geo = np.ascontiguousarray(geo.transpose(1, 0, 2)).reshape(P, 4 * P)

    adj8 = np.asarray(adj, dtype=np.int8).astype(f8)
    in_maps = []
    for c in range(NCORES):
        g = 1.0 if c < 4 else 0.0
        gcol = np.empty((P, 4), np.float32)
        gcol[:, 0] = g * SC
        gcol[:, 1] = (1.0 - g) * SC
        gcol[:, 2] = 0.0
        gcol[:, 3] = 1.0 - g
        selg = np.zeros((P, MB, 32), np.float32)
        for bp in range(MB):
            selg[:, bp, (c * MB + bp) % 32] = g
        # adjt[mb, p, jc*128+i] = adj[c*1024 + mb*128 + i, jc*128 + p]
        A = adj8[c * RPC:(c + 1) * RPC, :]
        adjt = np.ascontiguousarray(
            A.reshape(MB, P, NB, P).transpose(0, 3, 2, 1)).reshape(
                MB, P, NB * P)
        in_maps.append({
            "xt": xt,
            "wrhs": wrhs,
            "geo": geo,
            "adjt": adjt,
            "gcol": gcol,
            "selg": selg,
        })
    return in_maps


def kernel(x, weight, att_vec, adj, _trace=False, _trace_kwargs=None):
    nc = _get_program()
    in_maps = make_in_maps(x, weight, att_vec, adj)
    r = run_bass_kernel_spmd(nc, in_maps, core_ids=list(range(NCORES)),
                             trace=_trace, **(_trace_kwargs or {}))
    y = np.concatenate([r.results[c]["y"] for c in range(NCORES)], axis=0)
    kernel.last_results = r
    return y.astype(np.float32)


# revision 13
# speedup vs baseline: 1.0119x; 1.0119x over previous
"""GAT layer (nn_GAT_layer_67619965108552) as a Trainium2 Bass/Tile SPMD kernel.

Same collapsed math as the previous version (see prep_check.py for the numpy
emulation), restructured for speed:

  * The separate U-pass is gone: the score vectors ride as 6 extra rhs columns
    ([wam_hi | wam_lo] fp8 hi/lo split for precision) on the out-pass matmuls,
    so one pass over xT produces out AND [u, s1, s2] per node.
  * Even/odd pair extraction (tt = s1[even]+s2[odd], u pairs for the alphas)
    happens on the PE with a single resident 0/1 stationary E_eo: one N=24
    matmul per 8 blocks into one PSUM bank, whose columns line up so every
    downstream consumer is a uniform stride-6 2D slice. No DRAM roundtrip.
  * All DRAM operands are host-pre-tiled so every DMA descriptor is a 4-8 KB
    contiguous per-partition run (the old layouts were descriptor-rate bound
    at ~1 KB/descriptor).
  * Stage B runs in mb-slab order: adj arrives as 8 slabs of [all j, 128 i],
    each slab's 64 matmuls accumulate Y1/Y2 for one output row-block, and the
    per-block epilogue (alpha combine, normalize, sigmoid, store) overlaps the
    next slab's matmuls. No epilogue tail.
  * Stage B is all-fp8 (adj 0/1 exact; fo = f*out pre-scaled by 1/4 so the
    normalizer ratio is unchanged and values stay far below the TRN fp8e4
    240 cap). Numpy-emulated end-to-end rel err: 2.5e-3.

Sharding: rows of adj (and the output) across 8 cores, 1024 rows each;
x/weight replicated; every core computes the full out = x@W.
"""
import numpy as np

import concourse.bass as bass
import concourse.tile as tile
from concourse import bacc, mybir
from concourse.bass_utils import run_bass_kernel_spmd

F32 = mybir.dt.float32
F32R = mybir.dt.float32r
BF16 = mybir.dt.bfloat16
F8 = mybir.dt.float8e4

N = 8192
FIN = 512
FOUT = 256
P = 128
NB = N // P        # 64 node blocks
NCORES = 8
RPC = N // NCORES  # 1024 output rows per core
MB = RPC // P      # 8 output row blocks per core
NS = 8             # xT slices
SC = 0.25          # fo pre-scale (cancels in the normalizer ratio)


def build_program():
    nc = bacc.Bacc("TRN2", target_bir_lowering=False, debug=False,
                   num_devices=NCORES)

    xt_d = nc.dram_tensor("xt", [NS, P, 4 * 1024], F8, kind="ExternalInput")
    wrhs_d = nc.dram_tensor("wrhs", [P, 4 * 262], F8, kind="ExternalInput")
    geo_d = nc.dram_tensor("geo", [P, 4 * P], F32R, kind="ExternalInput")
    adjt_d = nc.dram_tensor("adjt", [MB, P, NB * P], F8, kind="ExternalInput")
    # gcol cols: [g*SC, (1-g)*SC, 0, 1-g]
    g_d = nc.dram_tensor("gcol", [P, 4], F32, kind="ExternalInput")
    # selg[p, bp, B] = g * (B == 8c + bp)
    selg_d = nc.dram_tensor("selg", [P, MB, 32], F32, kind="ExternalInput")
    y_d = nc.dram_tensor("y", [RPC, FOUT], F32, kind="ExternalOutput")

    with tile.TileContext(nc) as tc:
        with tc.tile_pool(name="const", bufs=1) as constp, \
             tc.tile_pool(name="adjp", bufs=3) as adjp, \
             tc.tile_pool(name="sa", bufs=3) as sa:

            # ---- constants (scalar HWDGE ring; sync ring streams xT) ----
            wrhs = constp.tile([P, 4, 262], F8)
            nc.scalar.dma_start(wrhs[:], wrhs_d.ap().rearrange(
                "p (c v) -> p c v", v=262))
            geo = constp.tile([P, 4, P], F32R)
            nc.scalar.dma_start(geo[:], geo_d.ap().rearrange(
                "p (g q) -> p g q", q=P))
            gcol = constp.tile([P, 4], F32)
            nc.scalar.dma_start(gcol[:], g_d.ap())
            selg = constp.tile([P, MB, 32], F32)
            nc.scalar.dma_start(selg[:], selg_d.ap())

            dumm = constp.tile([1, 2], F32)
            nc.gpsimd.memset(dumm[:], 0.0)

            # persistent staging: out blocks (col 256 preset to 1.0 so the
            # f-scale pass emits the normalizer column for free)
            outb = [constp.tile([P, FOUT + 1], BF16, name=f"outb{b}")
                    for b in range(NB)]
            for b in range(NB):
                nc.gpsimd.memset(outb[b][:, FOUT:FOUT + 1], 1.0)
            fo8 = [constp.tile([P, FOUT + 1], F8, name=f"fo8_{b}")
                   for b in range(NB)]
            uc6 = constp.tile([P, NB, 6], F32)
            ucols = constp.tile([P, NB, 3], F32R)
            fpm = constp.tile([P, 32], F32)
            al1 = constp.tile([P, MB], F32)
            al2 = constp.tile([P, MB], F32)

            # ---- fused out+score pass over the xT stream ----
            with tc.tile_pool(name="xtp", bufs=NS) as xtp, \
                 tc.tile_pool(name="ps_o", bufs=4, space="PSUM") as ps_o, \
                 tc.tile_pool(name="ps_e", bufs=1, space="PSUM") as ps_e:

                # eoX cols: [tts 32 | ae 32 | be 32], partition-aligned
                eoX = ps_e.tile([P, 96], F32, name="eoX")

                def extract_half(h):
                    # rhs: even/odd blocks of half h; dst cols 16h:16h+16
                    ev = ucols[:, 32 * h:32 * h + 32:2, :]
                    od = ucols[:, 32 * h + 1:32 * h + 32:2, :]
                    for base, pairs in ((0, ((0, ev, 1), (1, ev, 2),
                                             (2, od, 1), (3, od, 2))),
                                        (32, ((0, ev, 0), (2, od, 0))),
                                        (64, ((1, ev, 0), (3, od, 0)))):
                        dst = eoX[:, base + 16 * h:base + 16 * h + 16]
                        for k, (g, rr, v) in enumerate(pairs):
                            nc.tensor.matmul(dst, geo[:, g, :], rr[:, :, v],
                                             start=(k == 0),
                                             stop=(k == len(pairs) - 1))

                def fpm_half(h):
                    hs = slice(16 * h, 16 * h + 16)
                    lrt = sa.tile([P, 16], F32, tag="lrt", name="lrt")
                    nc.vector.tensor_scalar_mul(lrt[:], eoX[:, hs], 0.01)
                    nc.vector.tensor_max(lrt[:], eoX[:, hs], lrt[:])
                    ext = sa.tile([P, 16], F32, tag="ext", name="ext")
                    nc.scalar.activation(ext[:], lrt[:],
                                         mybir.ActivationFunctionType.Exp)
                    nc.vector.tensor_scalar(fpm[:, hs], ext[:], gcol[:, 1:2],
                                            gcol[:, 0:1],
                                            op0=mybir.AluOpType.mult,
                                            op1=mybir.AluOpType.add)

                def scale_one(jc):
                    kc = jc % 32
                    if jc % 2 == 0:
                        nc.scalar.activation(
                            fo8[jc][:], outb[jc][:],
                            mybir.ActivationFunctionType.Copy,
                            scale=fpm[:, kc:kc + 1])
                    else:
                        nc.vector.tensor_scalar_mul(fo8[jc][:], outb[jc][:],
                                                    fpm[:, kc:kc + 1])

                def half_jcs(h):
                    return list(range(16 * h, 16 * h + 16)) + \
                        list(range(32 + 16 * h, 32 + 16 * h + 16))

                pend_scale = []
                for s in range(NS):
                    if s == 5:
                        pend_scale = [jc for jc in half_jcs(0)
                                      if not 40 <= jc <= 47]
                    xs = xtp.tile([P, 4096], F8, tag="xts", name=f"xt{s}")
                    nc.sync.dma_start(xs[:], xt_d.ap()[s])
                    for b8 in range(8):
                        b = s * 8 + b8
                        po = ps_o.tile([P, 262], F32, tag="po", name="po")
                        for c in range(4):
                            nc.tensor.matmul(
                                po[:],
                                xs[:, c * 1024 + b8 * P:
                                   c * 1024 + (b8 + 1) * P],
                                wrhs[:, c, :], start=(c == 0), stop=(c == 3))
                        if b == 1:
                            # exp-table warm, anchored mid-stream
                            nc.scalar.activation(
                                dumm[:, 0:1], ucols[0:1, 0, 0:1],
                                mybir.ActivationFunctionType.Exp)
                        if b == 34:
                            # first-half extraction rides mid-stream (deps
                            # on slices 0-3 are settled by now)
                            extract_half(0)
                            fpm_half(0)
                        if b >= 36 and pend_scale:
                            # drip fo8 scales through the copy stream so the
                            # FIFO ACT/DVE queues never stall the PE
                            scale_one(pend_scale.pop(0))
                        if 40 <= b <= 47:
                            # kc = b-32 is in fpm half 0 (ready by now):
                            # fuse the f-scale into the PSUM drain, skip
                            # outb staging and the separate scale op
                            kc = b - 32
                            if b % 2 == 0:
                                nc.scalar.activation(
                                    fo8[b][:, 0:FOUT], po[:, 0:FOUT],
                                    mybir.ActivationFunctionType.Copy,
                                    scale=fpm[:, kc:kc + 1])
                                nc.vector.tensor_copy(uc6[:, b, :],
                                                      po[:, 256:262])
                                nc.vector.tensor_copy(fo8[b][:, FOUT:],
                                                      fpm[:, kc:kc + 1])
                            else:
                                nc.vector.tensor_scalar_mul(
                                    fo8[b][:, 0:FOUT], po[:, 0:FOUT],
                                    fpm[:, kc:kc + 1])
                                nc.scalar.copy(uc6[:, b, :], po[:, 256:262])
                                nc.scalar.copy(fo8[b][:, FOUT:],
                                               fpm[:, kc:kc + 1])
                        elif b % 2 == 0:
                            nc.scalar.copy(outb[b][:, 0:FOUT], po[:, 0:FOUT])
                            nc.vector.tensor_copy(uc6[:, b, :],
                                                  po[:, 256:262])
                        else:
                            nc.vector.tensor_copy(outb[b][:, 0:FOUT],
                                                  po[:, 0:FOUT])
                            nc.scalar.copy(uc6[:, b, :], po[:, 256:262])
                    sl = slice(s * 8, (s + 1) * 8)
                    nc.vector.tensor_add(ucols[:, sl, :], uc6[:, sl, 0:3],
                                         uc6[:, sl, 3:6])
                for jc in pend_scale:
                    scale_one(jc)
                extract_half(1)
                fpm_half(1)
                for jc in half_jcs(1):
                    scale_one(jc)
                # sigmoid-table warm, anchored on fpm
                nc.scalar.activation(dumm[:, 1:2], fpm[0:1, 0:1],
                                     mybir.ActivationFunctionType.Sigmoid)

                # alphas: aev/bev = exp(lrelu(ae/be)), al = selg-reduce + 1-g
                lra = sa.tile([P, 64], F32, tag="lra", name="lra")
                aeb = sa.tile([P, 64], F32, tag="aeb", name="aeb")
                nc.vector.tensor_copy(aeb[:, 0:32], eoX[:, 32:64])
                nc.vector.tensor_copy(aeb[:, 32:64], eoX[:, 64:96])
                nc.vector.tensor_scalar_mul(lra[:], aeb[:], 0.01)
                nc.vector.tensor_max(lra[:], aeb[:], lra[:])
                nc.scalar.activation(aeb[:], lra[:],
                                     mybir.ActivationFunctionType.Exp)
                for bp in range(MB):
                    m1 = sa.tile([P, 32], F32, tag="alm", name="alm1")
                    nc.vector.tensor_mul(m1[:], aeb[:, 0:32], selg[:, bp, :])
                    nc.vector.tensor_reduce(al1[:, bp:bp + 1], m1[:],
                                            axis=mybir.AxisListType.X,
                                            op=mybir.AluOpType.add)
                    m2 = sa.tile([P, 32], F32, tag="alm", name="alm2")
                    nc.vector.tensor_mul(m2[:], aeb[:, 32:64], selg[:, bp, :])
                    nc.vector.tensor_reduce(al2[:, bp:bp + 1], m2[:],
                                            axis=mybir.AxisListType.X,
                                            op=mybir.AluOpType.add)
                nc.vector.tensor_scalar_add(al1[:], al1[:], gcol[:, 3:4])
                nc.vector.tensor_scalar_add(al2[:], al2[:], gcol[:, 3:4])

            # ---- f-scale pass (jc order = stage-B consumption order) ----
            with tc.tile_pool(name="ps_y", bufs=4, space="PSUM") as ps_y, \
                 tc.tile_pool(name="comb", bufs=3) as comb:

                # ---- adj slabs (SWDGE; early ones gated on out-pass
                # progress so they don't steal HBM from the xT stream) ----
                def slab_load(mb, marker=None):
                    t = adjp.tile([P, NB * P], F8, tag="slab",
                                  name=f"slab{mb}")
                    if marker is not None:
                        nc.scalar.copy(t[0:1, 0:1], marker)
                    nc.gpsimd.dma_start(t[:], adjt_d.ap()[mb])
                    return t

                slabs = [slab_load(0, ucols[0:1, 8, 0:1]),
                         slab_load(1, ucols[0:1, 24, 0:1]),
                         slab_load(2, ucols[0:1, 40, 0:1])]
                slabs += [slab_load(mb) for mb in range(3, MB)]

                # ---- stage B: per row-block accumulate + inline epilogue ----
                for mb in range(MB):
                    at = slabs[mb]
                    ya = ps_y.tile([P, FOUT + 1], F32, tag="ya", name="ya")
                    yb = ps_y.tile([P, FOUT + 1], F32, tag="yb", name="yb")
                    for jc in range(NB):
                        dst = ya if jc < 32 else yb
                        nc.tensor.matmul(dst[:], at[:, jc * P:(jc + 1) * P], fo8[jc][:],
                                         start=(jc % 32 == 0),
                                         stop=(jc % 32 == 31))
                    za = comb.tile([P, FOUT + 1], F32, tag="za", name="za")
                    nc.scalar.activation(za[:], ya[:],
                                         mybir.ActivationFunctionType.Copy,
                                         scale=al1[:, mb:mb + 1])
                    t2 = comb.tile([P, FOUT + 1], F32, tag="t2", name="t2")
                    nc.vector.tensor_scalar_mul(t2[:], yb[:],
                                                al2[:, mb:mb + 1])
                    z2 = comb.tile([P, FOUT + 1], F32, tag="z2", name="z2")
                    nc.vector.tensor_add(z2[:], za[:], t2[:])
                    rec = comb.tile([P, 1], F32, tag="rec", name="rec")
                    nc.vector.reciprocal(rec[:], z2[:, FOUT:FOUT + 1])
                    res = comb.tile([P, FOUT], F32, tag="res", name="res")
                    nc.vector.tensor_scalar_mul(res[:], z2[:, :FOUT], rec[:])
                    resg = comb.tile([P, FOUT], F32, tag="resg", name="resg")
                    nc.scalar.activation(resg[:], res[:],
                                         mybir.ActivationFunctionType.Sigmoid)
                    nc.sync.dma_start(y_d.ap()[mb * P:(mb + 1) * P, :],
                                      resg[:])

    nc.compile()
    return nc


_NC_CACHE = None


def _get_program():
    global _NC_CACHE
    if _NC_CACHE is None:
        _NC_CACHE = build_program()
    return _NC_CACHE


def make_in_maps(x, weight, att_vec, adj):
    import ml_dtypes
    f8 = ml_dtypes.float8_e4m3
    x = np.asarray(x, dtype=np.float32)
    weight = np.ascontiguousarray(np.asarray(weight, dtype=np.float32))
    att_vec = np.asarray(att_vec, dtype=np.float32)

    # xt[s, p, c*1024+n] = x[s*1024+n, c*128+p]
    x8 = x.astype(f8)
    xt = np.ascontiguousarray(
        x8.reshape(NS, 1024, 4, P).transpose(0, 3, 2, 1)).reshape(NS, P, 4096)

    a1 = att_vec[:FOUT, 0]
    a2 = att_vec[FOUT:, 0]
    att3 = np.stack([a1 + a2, a1, a2], axis=1).astype(np.float32)
    wam = (weight @ att3).astype(np.float32)            # [FIN, 3]
    wam_hi = wam.astype(f8)
    wam_lo = (wam - wam_hi.astype(np.float32)).astype(f8)
    wcat = np.concatenate([weight.astype(f8),
                           wam_hi, wam_lo], axis=1)     # [FIN, 262]
    wrhs = np.ascontiguousarray(
        wcat.reshape(4, P, 262).transpose(1, 0, 2)).reshape(P, 4 * 262)

    geo = np.zeros((4, P, P), np.float32)   # [g, p, q]
    q = np.arange(64)
    geo[0, 2 * q, q] = 1.0          # even -> top
    geo[1, 2 * q + 1, q] = 1.0      # odd  -> top
    geo[2, 2 * q, q + 64] = 1.0     # even -> bot
    geo[3, 2 * q + 1, q + 64] = 1.0  # odd -> bot
    geo = np.ascontiguousarray(geo.transpose(1, 0, 2)).reshape(P, 4 * P)

    adj8 = np.asarray(adj, dtype=np.int8).astype(f8)
    in_maps = []
    for c in range(NCORES):
        g = 1.0 if c < 4 else 0.0
        gcol = np.empty((P, 4), np.float32)
        gcol[:, 0] = g * SC
        gcol[:, 1] = (1.0 - g) * SC
        gcol[:, 2] = 0.0
        gcol[:, 3] = 1.0 - g
        selg = np.zeros((P, MB, 32), np.float32)
        for bp in range(MB):
            selg[:, bp, (c * MB + bp) % 32] = g
        # adjt[mb, p, jc*128+i] = adj[c*1024 + mb*128 + i, jc*128 + p]
        A = adj8[c * RPC:(c + 1) * RPC, :]
        adjt = np.ascontiguousarray(
            A.reshape(MB, P, NB, P).transpose(0, 3, 2, 1)).reshape(
                MB, P, NB * P)
        in_maps.append({
            "xt": xt,
            "wrhs": wrhs,
            "geo": geo,
            "adjt": adjt,
            "gcol": gcol,
            "selg": selg,
        })
    return in_maps


def kernel(x, weight, att_vec, adj, _trace=False, _trace_kwargs=None):
    nc = _get_program()
    in_maps = make_in_maps(x, weight, att_vec, adj)
    r = run_bass_kernel_spmd(nc, in_maps, core_ids=list(range(NCORES)),
                             trace=_trace, **(_trace_kwargs or {}))
    y = np.concatenate([r.results[c]["y"] for c in range(NCORES)], axis=0)
    kernel.last_results = r
    return y.astype(np.float32)


# revision 14
# speedup vs baseline: 1.1582x; 1.1446x over previous
"""GAT layer (nn_GAT_layer_67619965108552) as a Trainium2 Bass/Tile SPMD kernel.

Same collapsed math as the previous version (see prep_check.py for the numpy
emulation), restructured for speed:

  * The separate U-pass is gone: the score vectors ride as 6 extra rhs columns
    ([wam_hi | wam_lo] fp8 hi/lo split for precision) on the out-pass matmuls,
    so one pass over xT produces out AND [u, s1, s2] per node.
  * Even/odd pair extraction (tt = s1[even]+s2[odd], u pairs for the alphas)
    happens on the PE with a single resident 0/1 stationary E_eo: one N=24
    matmul per 8 blocks into one PSUM bank, whose columns line up so every
    downstream consumer is a uniform stride-6 2D slice. No DRAM roundtrip.
  * All DRAM operands are host-pre-tiled so every DMA descriptor is a 4-8 KB
    contiguous per-partition run (the old layouts were descriptor-rate bound
    at ~1 KB/descriptor).
  * Stage B runs in mb-slab order: adj arrives as 8 slabs of [all j, 128 i],
    each slab's 64 matmuls accumulate Y1/Y2 for one output row-block, and the
    per-block epilogue (alpha combine, normalize, sigmoid, store) overlaps the
    next slab's matmuls. No epilogue tail.
  * Stage B is all-fp8 (adj 0/1 exact; fo = f*out pre-scaled by 1/4 so the
    normalizer ratio is unchanged and values stay far below the TRN fp8e4
    240 cap). Numpy-emulated end-to-end rel err: 2.5e-3.

Sharding: rows of adj (and the output) across 8 cores, 1024 rows each;
x/weight replicated; every core computes the full out = x@W.
"""
import numpy as np

import concourse.bass as bass
import concourse.tile as tile
from concourse import bacc, mybir
from concourse.bass_utils import run_bass_kernel_spmd

F32 = mybir.dt.float32
F32R = mybir.dt.float32r
BF16 = mybir.dt.bfloat16
F8 = mybir.dt.float8e4

N = 8192
FIN = 512
FOUT = 256
P = 128
NB = N // P        # 64 node blocks
NCORES = 8
RPC = N // NCORES  # 1024 output rows per core
MB = RPC // P      # 8 output row blocks per core
NS = 8             # xT slices
SC = 0.25          # fo pre-scale (cancels in the normalizer ratio)


def build_program():
    nc = bacc.Bacc("TRN2", target_bir_lowering=False, debug=False,
                   num_devices=NCORES)

    xt_d = nc.dram_tensor("xt", [NS, P, 4 * 1024], F8, kind="ExternalInput")
    wrhs_d = nc.dram_tensor("wrhs", [P, 4 * 262], F8, kind="ExternalInput")
    geo_d = nc.dram_tensor("geo", [P, 4 * P], F32R, kind="ExternalInput")
    adjt_d = nc.dram_tensor("adjt", [MB, P, NB * P], F8, kind="ExternalInput")
    # gcol cols: [g*SC, (1-g)*SC, 0, 1-g]
    g_d = nc.dram_tensor("gcol", [P, 4], F32, kind="ExternalInput")
    # selg[p, bp, B] = g * (B == 8c + bp)
    selg_d = nc.dram_tensor("selg", [P, MB, 32], F32, kind="ExternalInput")
    y_d = nc.dram_tensor("y", [RPC, FOUT], F32, kind="ExternalOutput")

    with tile.TileContext(nc) as tc:
        with tc.tile_pool(name="const", bufs=1) as constp, \
             tc.tile_pool(name="adjp", bufs=3) as adjp, \
             tc.tile_pool(name="sa", bufs=3) as sa:

            # ---- constants (scalar HWDGE ring; sync ring streams xT) ----
            wrhs = constp.tile([P, 4, 262], F8)
            nc.sync.dma_start(wrhs[:], wrhs_d.ap().rearrange(
                "p (c v) -> p c v", v=262))
            geo = constp.tile([P, 4, P], F32R)
            nc.scalar.dma_start(geo[:], geo_d.ap().rearrange(
                "p (g q) -> p g q", q=P))
            gcol = constp.tile([P, 4], F32)
            nc.scalar.dma_start(gcol[:], g_d.ap())
            selg = constp.tile([P, MB, 32], F32)
            nc.scalar.dma_start(selg[:], selg_d.ap())

            dumm = constp.tile([1, 2], F32)
            nc.gpsimd.memset(dumm[:], 0.0)

            # persistent staging: out blocks (col 256 preset to 1.0 so the
            # f-scale pass emits the normalizer column for free)
            outb = [constp.tile([P, FOUT + 1], BF16, name=f"outb{b}")
                    for b in range(NB)]
            for b in range(NB):
                nc.gpsimd.memset(outb[b][:, FOUT:FOUT + 1], 1.0)
            fo8 = [constp.tile([P, FOUT + 1], F8, name=f"fo8_{b}")
                   for b in range(NB)]
            uc6 = constp.tile([P, NB, 6], F32)
            ucols = constp.tile([P, NB, 3], F32R)
            fpm = constp.tile([P, 32], F32)
            al1 = constp.tile([P, MB], F32)
            al2 = constp.tile([P, MB], F32)

            # ---- fused out+score pass over the xT stream ----
            with tc.tile_pool(name="xtp", bufs=NS) as xtp, \
                 tc.tile_pool(name="ps_o", bufs=4, space="PSUM") as ps_o, \
                 tc.tile_pool(name="ps_e", bufs=1, space="PSUM") as ps_e:

                # eoX cols: [tts 32 | ae 32 | be 32], partition-aligned
                eoX = ps_e.tile([P, 96], F32, name="eoX")

                def extract_half(lo, hi):
                    # rhs: even/odd blocks for kc in [lo, hi)
                    ev = ucols[:, 2 * lo:2 * hi:2, :]
                    od = ucols[:, 2 * lo + 1:2 * hi:2, :]
                    for base, pairs in ((0, ((0, ev, 1), (1, ev, 2),
                                             (2, od, 1), (3, od, 2))),
                                        (32, ((0, ev, 0), (2, od, 0))),
                                        (64, ((1, ev, 0), (3, od, 0)))):
                        dst = eoX[:, base + lo:base + hi]
                        for k, (g, rr, v) in enumerate(pairs):
                            nc.tensor.matmul(dst, geo[:, g, :], rr[:, :, v],
                                             start=(k == 0),
                                             stop=(k == len(pairs) - 1))

                def fpm_half(lo, hi):
                    hs = slice(lo, hi)
                    lrt = sa.tile([P, hi - lo], F32, tag="lrt", name="lrt")
                    nc.vector.tensor_scalar_mul(lrt[:], eoX[:, hs], 0.01)
                    nc.vector.tensor_max(lrt[:], eoX[:, hs], lrt[:])
                    ext = sa.tile([P, hi - lo], F32, tag="ext", name="ext")
                    nc.scalar.activation(ext[:], lrt[:],
                                         mybir.ActivationFunctionType.Exp)
                    nc.vector.tensor_scalar(fpm[:, hs], ext[:], gcol[:, 1:2],
                                            gcol[:, 0:1],
                                            op0=mybir.AluOpType.mult,
                                            op1=mybir.AluOpType.add)

                def scale_one(jc):
                    kc = jc % 32
                    if jc % 2 == 0:
                        nc.scalar.activation(
                            fo8[jc][:], outb[jc][:],
                            mybir.ActivationFunctionType.Copy,
                            scale=fpm[:, kc:kc + 1])
                    else:
                        nc.vector.tensor_scalar_mul(fo8[jc][:], outb[jc][:],
                                                    fpm[:, kc:kc + 1])

                def half_jcs(h):
                    return list(range(16 * h, 16 * h + 16)) + \
                        list(range(32 + 16 * h, 32 + 16 * h + 16))

                pend_scale = []
                for s in range(NS):
                    if s == 5:
                        pend_scale = [jc for jc in half_jcs(0)
                                      if not 40 <= jc <= 47]
                    xs = xtp.tile([P, 4096], F8, tag="xts", name=f"xt{s}")
                    nc.sync.dma_start(xs[:], xt_d.ap()[s])
                    for b8 in range(8):
                        b = s * 8 + b8
                        po = ps_o.tile([P, 262], F32, tag="po", name="po")
                        for c in range(4):
                            nc.tensor.matmul(
                                po[:],
                                xs[:, c * 1024 + b8 * P:
                                   c * 1024 + (b8 + 1) * P],
                                wrhs[:, c, :], start=(c == 0), stop=(c == 3))
                        if b == 1:
                            # exp-table warm, anchored mid-stream
                            nc.scalar.activation(
                                dumm[:, 0:1], ucols[0:1, 0, 0:1],
                                mybir.ActivationFunctionType.Exp)
                        if b == 34:
                            # first-quarter extraction rides mid-stream
                            # (deps on slices 0-3 are settled by now)
                            extract_half(0, 16)
                            fpm_half(0, 16)
                        if b == 50:
                            extract_half(16, 24)
                            fpm_half(16, 24)
                        if b == 52:
                            pend_scale += [jc for jc in range(16, 24)] + \
                                [jc for jc in range(48, 56)]
                        if b >= 36 and pend_scale:
                            # drip fo8 scales through the copy stream so the
                            # FIFO ACT/DVE queues never stall the PE
                            scale_one(pend_scale.pop(0))
                        if 40 <= b <= 47:
                            # kc = b-32 is in fpm half 0 (ready by now):
                            # fuse the f-scale into the PSUM drain, skip
                            # outb staging and the separate scale op
                            kc = b - 32
                            if b % 2 == 0:
                                nc.scalar.activation(
                                    fo8[b][:, 0:FOUT], po[:, 0:FOUT],
                                    mybir.ActivationFunctionType.Copy,
                                    scale=fpm[:, kc:kc + 1])
                                nc.vector.tensor_copy(uc6[:, b, :],
                                                      po[:, 256:262])
                                nc.vector.tensor_copy(fo8[b][:, FOUT:],
                                                      fpm[:, kc:kc + 1])
                            else:
                                nc.vector.tensor_scalar_mul(
                                    fo8[b][:, 0:FOUT], po[:, 0:FOUT],
                                    fpm[:, kc:kc + 1])
                                nc.scalar.copy(uc6[:, b, :], po[:, 256:262])
                                nc.scalar.copy(fo8[b][:, FOUT:],
                                               fpm[:, kc:kc + 1])
                        elif b % 2 == 0:
                            nc.scalar.copy(outb[b][:, 0:FOUT], po[:, 0:FOUT])
                            nc.vector.tensor_copy(uc6[:, b, :],
                                                  po[:, 256:262])
                        else:
                            nc.vector.tensor_copy(outb[b][:, 0:FOUT],
                                                  po[:, 0:FOUT])
                            nc.scalar.copy(uc6[:, b, :], po[:, 256:262])
                    sl = slice(s * 8, (s + 1) * 8)
                    nc.vector.tensor_add(ucols[:, sl, :], uc6[:, sl, 0:3],
                                         uc6[:, sl, 3:6])
                extract_half(24, 32)
                fpm_half(24, 32)
                for jc in pend_scale + list(range(24, 32)) + \
                        list(range(56, 64)):
                    scale_one(jc)
                # sigmoid-table warm, anchored on fpm
                nc.scalar.activation(dumm[:, 1:2], fpm[0:1, 0:1],
                                     mybir.ActivationFunctionType.Sigmoid)

                # alphas: aev/bev = exp(lrelu(ae/be)), al = selg-reduce + 1-g
                lra = sa.tile([P, 64], F32, tag="lra", name="lra")
                aeb = sa.tile([P, 64], F32, tag="aeb", name="aeb")
                nc.vector.tensor_copy(aeb[:, 0:32], eoX[:, 32:64])
                nc.vector.tensor_copy(aeb[:, 32:64], eoX[:, 64:96])
                nc.vector.tensor_scalar_mul(lra[:], aeb[:], 0.01)
                nc.vector.tensor_max(lra[:], aeb[:], lra[:])
                nc.scalar.activation(aeb[:], lra[:],
                                     mybir.ActivationFunctionType.Exp)
                for bp in range(MB):
                    m1 = sa.tile([P, 32], F32, tag="alm", name="alm1")
                    nc.vector.tensor_mul(m1[:], aeb[:, 0:32], selg[:, bp, :])
                    nc.vector.tensor_reduce(al1[:, bp:bp + 1], m1[:],
                                            axis=mybir.AxisListType.X,
                                            op=mybir.AluOpType.add)
                    m2 = sa.tile([P, 32], F32, tag="alm", name="alm2")
                    nc.vector.tensor_mul(m2[:], aeb[:, 32:64], selg[:, bp, :])
                    nc.vector.tensor_reduce(al2[:, bp:bp + 1], m2[:],
                                            axis=mybir.AxisListType.X,
                                            op=mybir.AluOpType.add)
                nc.vector.tensor_scalar_add(al1[:], al1[:], gcol[:, 3:4])
                nc.vector.tensor_scalar_add(al2[:], al2[:], gcol[:, 3:4])

            # ---- f-scale pass (jc order = stage-B consumption order) ----
            with tc.tile_pool(name="ps_y", bufs=4, space="PSUM") as ps_y, \
                 tc.tile_pool(name="comb", bufs=3) as comb:

                # ---- adj slabs (SWDGE; early ones gated on out-pass
                # progress so they don't steal HBM from the xT stream) ----
                def slab_load(mb, marker=None):
                    t = adjp.tile([P, NB * P], F8, tag="slab",
                                  name=f"slab{mb}")
                    if marker is not None:
                        nc.scalar.copy(t[0:1, 0:1], marker)
                    nc.gpsimd.dma_start(t[:], adjt_d.ap()[mb])
                    return t

                slabs = [slab_load(0, ucols[0:1, 8, 0:1]),
                         slab_load(1, ucols[0:1, 24, 0:1]),
                         slab_load(2, ucols[0:1, 40, 0:1])]
                slabs += [slab_load(mb) for mb in range(3, MB)]

                # ---- stage B: per row-block accumulate + inline epilogue ----
                for mb in range(MB):
                    at = slabs[mb]
                    ya = ps_y.tile([P, FOUT + 1], F32, tag="ya", name="ya")
                    yb = ps_y.tile([P, FOUT + 1], F32, tag="yb", name="yb")
                    for jc in range(NB):
                        dst = ya if jc < 32 else yb
                        nc.tensor.matmul(dst[:], at[:, jc * P:(jc + 1) * P], fo8[jc][:],
                                         start=(jc % 32 == 0),
                                         stop=(jc % 32 == 31))
                    za = comb.tile([P, FOUT + 1], F32, tag="za", name="za")
                    nc.scalar.activation(za[:], ya[:],
                                         mybir.ActivationFunctionType.Copy,
                                         scale=al1[:, mb:mb + 1])
                    t2 = comb.tile([P, FOUT + 1], F32, tag="t2", name="t2")
                    nc.vector.tensor_scalar_mul(t2[:], yb[:],
                                                al2[:, mb:mb + 1])
                    z2 = comb.tile([P, FOUT + 1], F32, tag="z2", name="z2")
                    nc.vector.tensor_add(z2[:], za[:], t2[:])
                    rec = comb.tile([P, 1], F32, tag="rec", name="rec")
                    nc.vector.reciprocal(rec[:], z2[:, FOUT:FOUT + 1])
                    res = comb.tile([P, FOUT], F32, tag="res", name="res")
                    resg = comb.tile([P, FOUT], F32, tag="resg", name="resg")
                    for hh in range(2):
                        cs = slice(hh * 128, hh * 128 + 128)
                        nc.vector.tensor_scalar_mul(res[:, cs], z2[:, cs],
                                                    rec[:])
                        nc.scalar.activation(
                            resg[:, cs], res[:, cs],
                            mybir.ActivationFunctionType.Sigmoid)
                        nc.sync.dma_start(
                            y_d.ap()[mb * P:(mb + 1) * P, cs], resg[:, cs])

    nc.compile()
    return nc


_NC_CACHE = None


def _get_program():
    global _NC_CACHE
    if _NC_CACHE is None:
        _NC_CACHE = build_program()
    return _NC_CACHE


def make_in_maps(x, weight, att_vec, adj):
    import ml_dtypes
    f8 = ml_dtypes.float8_e4m3
    x = np.asarray(x, dtype=np.float32)
    weight = np.ascontiguousarray(np.asarray(weight, dtype=np.float32))
    att_vec = np.asarray(att_vec, dtype=np.float32)

    # xt[s, p, c*1024+n] = x[s*1024+n, c*128+p]
    x8 = x.astype(f8)
    xt = np.ascontiguousarray(
        x8.reshape(NS, 1024, 4, P).transpose(0, 3, 2, 1)).reshape(NS, P, 4096)

    a1 = att_vec[:FOUT, 0]
    a2 = att_vec[FOUT:, 0]
    att3 = np.stack([a1 + a2, a1, a2], axis=1).astype(np.float32)
    wam = (weight @ att3).astype(np.float32)            # [FIN, 3]
    wam_hi = wam.astype(f8)
    wam_lo = (wam - wam_hi.astype(np.float32)).astype(f8)
    wcat = np.concatenate([weight.astype(f8),
                           wam_hi, wam_lo], axis=1)     # [FIN, 262]
    wrhs = np.ascontiguousarray(
        wcat.reshape(4, P, 262).transpose(1, 0, 2)).reshape(P, 4 * 262)

    geo = np.zeros((4, P, P), np.float32)   # [g, p, q]
    q = np.arange(64)
    geo[0, 2 * q, q] = 1.0          # even -> top
    geo[1, 2 * q + 1, q] = 1.0      # odd  -> top
    geo[2, 2 * q, q + 64] = 1.0     # even -> bot
    geo[3, 2 * q + 1, q + 64] = 1.0  # odd -> bot
    geo = np.ascontiguousarray(geo.transpose(1, 0, 2)).reshape(P, 4 * P)

    adj8 = np.asarray(adj, dtype=np.int8).astype(f8)
    in_maps = []
    for c in range(NCORES):
        g = 1.0 if c < 4 else 0.0
        gcol = np.empty((P, 4), np.float32)
        gcol[:, 0] = g * SC
        gcol[:, 1] = (1.0 - g) * SC
        gcol[:, 2] = 0.0
        gcol[:, 3] = 1.0 - g
        selg = np.zeros((P, MB, 32), np.float32)
        for bp in range(MB):
            selg[:, bp, (c * MB + bp) % 32] = g
        # adjt[mb, p, jc*128+i] = adj[c*1024 + mb*128 + i, jc*128 + p]
        A = adj8[c * RPC:(c + 1) * RPC, :]
        adjt = np.ascontiguousarray(
            A.reshape(MB, P, NB, P).transpose(0, 3, 2, 1)).reshape(
                MB, P, NB * P)
        in_maps.append({
            "xt": xt,
            "wrhs": wrhs,
            "geo": geo,
            "adjt": adjt,
            "gcol": gcol,
            "selg": selg,
        })
    return in_maps


def kernel(x, weight, att_vec, adj, _trace=False, _trace_kwargs=None):
    nc = _get_program()
    in_maps = make_in_maps(x, weight, att_vec, adj)
    r = run_bass_kernel_spmd(nc, in_maps, core_ids=list(range(NCORES)),
                             trace=_trace, **(_trace_kwargs or {}))
    y = np.concatenate([r.results[c]["y"] for c in range(NCORES)], axis=0)
    kernel.last_results = r
    return y.astype(np.float32)


# revision 16
# speedup vs baseline: 1.1847x; 1.0228x over previous
"""GAT layer (nn_GAT_layer_67619965108552) as a Trainium2 Bass/Tile SPMD kernel.

Same collapsed math as the previous version (see prep_check.py for the numpy
emulation), restructured for speed:

  * The separate U-pass is gone: the score vectors ride as 6 extra rhs columns
    ([wam_hi | wam_lo] fp8 hi/lo split for precision) on the out-pass matmuls,
    so one pass over xT produces out AND [u, s1, s2] per node.
  * Even/odd pair extraction (tt = s1[even]+s2[odd], u pairs for the alphas)
    happens on the PE with a single resident 0/1 stationary E_eo: one N=24
    matmul per 8 blocks into one PSUM bank, whose columns line up so every
    downstream consumer is a uniform stride-6 2D slice. No DRAM roundtrip.
  * All DRAM operands are host-pre-tiled so every DMA descriptor is a 4-8 KB
    contiguous per-partition run (the old layouts were descriptor-rate bound
    at ~1 KB/descriptor).
  * Stage B runs in mb-slab order: adj arrives as 8 slabs of [all j, 128 i],
    each slab's 64 matmuls accumulate Y1/Y2 for one output row-block, and the
    per-block epilogue (alpha combine, normalize, sigmoid, store) overlaps the
    next slab's matmuls. No epilogue tail.
  * Stage B is all-fp8 (adj 0/1 exact; fo = f*out pre-scaled by 1/4 so the
    normalizer ratio is unchanged and values stay far below the TRN fp8e4
    240 cap). Numpy-emulated end-to-end rel err: 2.5e-3.

Sharding: rows of adj (and the output) across 8 cores, 1024 rows each;
x/weight replicated; every core computes the full out = x@W.
"""
import numpy as np

import concourse.bass as bass
import concourse.tile as tile
from concourse import bacc, mybir
from concourse.bass_utils import run_bass_kernel_spmd

F32 = mybir.dt.float32
F32R = mybir.dt.float32r
BF16 = mybir.dt.bfloat16
F8 = mybir.dt.float8e4

N = 8192
FIN = 512
FOUT = 256
P = 128
NB = N // P        # 64 node blocks
NCORES = 8
RPC = N // NCORES  # 1024 output rows per core
MB = RPC // P      # 8 output row blocks per core
NS = 8             # xT slices
SC = 0.25          # fo pre-scale (cancels in the normalizer ratio)


def build_program():
    nc = bacc.Bacc("TRN2", target_bir_lowering=False, debug=False,
                   num_devices=NCORES)

    xt_d = nc.dram_tensor("xt", [NS, P, 4 * 1024], F8, kind="ExternalInput")
    wrhs_d = nc.dram_tensor("wrhs", [P, 4 * 262], F8, kind="ExternalInput")
    geo_d = nc.dram_tensor("geo", [P, 4 * P], F32R, kind="ExternalInput")
    adjt_d = nc.dram_tensor("adjt", [MB, P, NB * P], F8, kind="ExternalInput")
    # gcol cols: [g*SC, (1-g)*SC, 0, 1-g]
    g_d = nc.dram_tensor("gcol", [P, 4], F32, kind="ExternalInput")
    # selg[p, bp, B] = g * (B == 8c + bp)
    selg_d = nc.dram_tensor("selg", [P, MB, 32], F32, kind="ExternalInput")
    y_d = nc.dram_tensor("y", [RPC, FOUT], F32, kind="ExternalOutput")

    with tile.TileContext(nc) as tc:
        with tc.tile_pool(name="const", bufs=1) as constp, \
             tc.tile_pool(name="adjp", bufs=3) as adjp, \
             tc.tile_pool(name="sa", bufs=3) as sa:

            # ---- constants (scalar HWDGE ring; sync ring streams xT) ----
            wrhs = constp.tile([P, 4, 262], F8)
            nc.scalar.dma_start(wrhs[:], wrhs_d.ap().rearrange(
                "p (c v) -> p c v", v=262))
            geo = constp.tile([P, 4, P], F32R)
            nc.scalar.dma_start(geo[:], geo_d.ap().rearrange(
                "p (g q) -> p g q", q=P))
            gcol = constp.tile([P, 4], F32)
            nc.scalar.dma_start(gcol[:], g_d.ap())
            selg = constp.tile([P, MB, 32], F32)
            nc.scalar.dma_start(selg[:], selg_d.ap())

            dumm = constp.tile([1, 2], F32)
            nc.gpsimd.memset(dumm[:], 0.0)

            # persistent staging: out blocks (col 256 preset to 1.0 so the
            # f-scale pass emits the normalizer column for free)
            outb = [constp.tile([P, FOUT + 1], BF16, name=f"outb{b}")
                    for b in range(NB)]
            for b in range(NB):
                nc.gpsimd.memset(outb[b][:, FOUT:FOUT + 1], 1.0)
            fo8 = [constp.tile([P, FOUT + 1], F8, name=f"fo8_{b}")
                   for b in range(NB)]
            uc6 = constp.tile([P, NB, 6], F32)
            ucols = constp.tile([P, NB, 3], F32R)
            fpm = constp.tile([P, 32], F32)
            al1 = constp.tile([P, MB], F32)
            al2 = constp.tile([P, MB], F32)

            # ---- fused out+score pass over the xT stream ----
            with tc.tile_pool(name="xtp", bufs=NS) as xtp, \
                 tc.tile_pool(name="ps_o", bufs=4, space="PSUM") as ps_o, \
                 tc.tile_pool(name="ps_e", bufs=1, space="PSUM") as ps_e:

                # eoX cols: [tts 32 | ae 32 | be 32], partition-aligned
                eoX = ps_e.tile([P, 96], F32, name="eoX")

                def extract_half(lo, hi):
                    # rhs: even/odd blocks for kc in [lo, hi)
                    ev = ucols[:, 2 * lo:2 * hi:2, :]
                    od = ucols[:, 2 * lo + 1:2 * hi:2, :]
                    for base, pairs in ((0, ((0, ev, 1), (1, ev, 2),
                                             (2, od, 1), (3, od, 2))),
                                        (32, ((0, ev, 0), (2, od, 0))),
                                        (64, ((1, ev, 0), (3, od, 0)))):
                        dst = eoX[:, base + lo:base + hi]
                        for k, (g, rr, v) in enumerate(pairs):
                            nc.tensor.matmul(dst, geo[:, g, :], rr[:, :, v],
                                             start=(k == 0),
                                             stop=(k == len(pairs) - 1))

                def fpm_half(lo, hi):
                    hs = slice(lo, hi)
                    lrt = sa.tile([P, hi - lo], F32, tag="lrt", name="lrt")
                    nc.vector.tensor_scalar_mul(lrt[:], eoX[:, hs], 0.01)
                    nc.vector.tensor_max(lrt[:], eoX[:, hs], lrt[:])
                    ext = sa.tile([P, hi - lo], F32, tag="ext", name="ext")
                    nc.scalar.activation(ext[:], lrt[:],
                                         mybir.ActivationFunctionType.Exp)
                    nc.vector.tensor_scalar(fpm[:, hs], ext[:], gcol[:, 1:2],
                                            gcol[:, 0:1],
                                            op0=mybir.AluOpType.mult,
                                            op1=mybir.AluOpType.add)

                def scale_one(jc):
                    kc = jc % 32
                    if jc % 2 == 0:
                        nc.scalar.activation(
                            fo8[jc][:], outb[jc][:],
                            mybir.ActivationFunctionType.Copy,
                            scale=fpm[:, kc:kc + 1])
                    else:
                        nc.vector.tensor_scalar_mul(fo8[jc][:], outb[jc][:],
                                                    fpm[:, kc:kc + 1])

                def half_jcs(h):
                    return list(range(16 * h, 16 * h + 16)) + \
                        list(range(32 + 16 * h, 32 + 16 * h + 16))

                pend_scale = []
                for s in range(NS):
                    if s == 5:
                        pend_scale = [jc for jc in half_jcs(0)
                                      if not 40 <= jc <= 47]
                    xs = xtp.tile([P, 4096], F8, tag="xts", name=f"xt{s}")
                    nc.sync.dma_start(xs[:], xt_d.ap()[s])
                    for b8 in range(8):
                        b = s * 8 + b8
                        po = ps_o.tile([P, 262], F32, tag="po", name="po")
                        for c in range(4):
                            nc.tensor.matmul(
                                po[:],
                                xs[:, c * 1024 + b8 * P:
                                   c * 1024 + (b8 + 1) * P],
                                wrhs[:, c, :], start=(c == 0), stop=(c == 3))
                        if b == 1:
                            # exp-table warm, anchored mid-stream
                            nc.scalar.activation(
                                dumm[:, 0:1], ucols[0:1, 0, 0:1],
                                mybir.ActivationFunctionType.Exp)
                        if b == 34:
                            # first-quarter extraction rides mid-stream
                            # (deps on slices 0-3 are settled by now)
                            extract_half(0, 16)
                            fpm_half(0, 16)
                        if b == 50:
                            extract_half(16, 24)
                            fpm_half(16, 24)
                        if b == 52:
                            pend_scale += [jc for jc in range(16, 24)] + \
                                [jc for jc in range(48, 56)]
                        if b >= 36 and pend_scale:
                            # drip fo8 scales through the copy stream so the
                            # FIFO ACT/DVE queues never stall the PE
                            scale_one(pend_scale.pop(0))
                        if 40 <= b <= 47:
                            # kc = b-32 is in fpm half 0 (ready by now):
                            # fuse the f-scale into the PSUM drain, skip
                            # outb staging and the separate scale op
                            kc = b - 32
                            if b % 2 == 0:
                                nc.scalar.activation(
                                    fo8[b][:, 0:FOUT], po[:, 0:FOUT],
                                    mybir.ActivationFunctionType.Copy,
                                    scale=fpm[:, kc:kc + 1])
                                nc.vector.tensor_copy(uc6[:, b, :],
                                                      po[:, 256:262])
                                nc.vector.tensor_copy(fo8[b][:, FOUT:],
                                                      fpm[:, kc:kc + 1])
                            else:
                                nc.vector.tensor_scalar_mul(
                                    fo8[b][:, 0:FOUT], po[:, 0:FOUT],
                                    fpm[:, kc:kc + 1])
                                nc.scalar.copy(uc6[:, b, :], po[:, 256:262])
                                nc.scalar.copy(fo8[b][:, FOUT:],
                                               fpm[:, kc:kc + 1])
                        elif b % 2 == 0:
                            nc.scalar.copy(outb[b][:, 0:FOUT], po[:, 0:FOUT])
                            nc.vector.tensor_copy(uc6[:, b, :],
                                                  po[:, 256:262])
                        else:
                            nc.vector.tensor_copy(outb[b][:, 0:FOUT],
                                                  po[:, 0:FOUT])
                            nc.scalar.copy(uc6[:, b, :], po[:, 256:262])
                    sl = slice(s * 8, (s + 1) * 8)
                    nc.vector.tensor_add(ucols[:, sl, :], uc6[:, sl, 0:3],
                                         uc6[:, sl, 3:6])
                extract_half(24, 32)
                fpm_half(24, 32)
                for jc in pend_scale + list(range(24, 32)) + \
                        list(range(56, 64)):
                    scale_one(jc)
                # sigmoid-table warm, anchored on fpm
                nc.scalar.activation(dumm[:, 1:2], fpm[0:1, 0:1],
                                     mybir.ActivationFunctionType.Sigmoid)

                # alphas: aev/bev = exp(lrelu(ae/be)), al = selg-reduce + 1-g
                lra = sa.tile([P, 64], F32, tag="lra", name="lra")
                aeb = sa.tile([P, 64], F32, tag="aeb", name="aeb")
                nc.vector.tensor_copy(aeb[:, 0:32], eoX[:, 32:64])
                nc.vector.tensor_copy(aeb[:, 32:64], eoX[:, 64:96])
                nc.vector.tensor_scalar_mul(lra[:], aeb[:], 0.01)
                nc.vector.tensor_max(lra[:], aeb[:], lra[:])
                nc.scalar.activation(aeb[:], lra[:],
                                     mybir.ActivationFunctionType.Exp)
                for bp in range(MB):
                    m1 = sa.tile([P, 32], F32, tag="alm", name="alm1")
                    nc.vector.tensor_mul(m1[:], aeb[:, 0:32], selg[:, bp, :])
                    nc.vector.tensor_reduce(al1[:, bp:bp + 1], m1[:],
                                            axis=mybir.AxisListType.X,
                                            op=mybir.AluOpType.add)
                    m2 = sa.tile([P, 32], F32, tag="alm", name="alm2")
                    nc.vector.tensor_mul(m2[:], aeb[:, 32:64], selg[:, bp, :])
                    nc.vector.tensor_reduce(al2[:, bp:bp + 1], m2[:],
                                            axis=mybir.AxisListType.X,
                                            op=mybir.AluOpType.add)
                nc.vector.tensor_scalar_add(al1[:], al1[:], gcol[:, 3:4])
                nc.vector.tensor_scalar_add(al2[:], al2[:], gcol[:, 3:4])

            # ---- f-scale pass (jc order = stage-B consumption order) ----
            with tc.tile_pool(name="ps_y", bufs=4, space="PSUM") as ps_y, \
                 tc.tile_pool(name="comb", bufs=3) as comb:

                # ---- adj slabs (SWDGE; early ones gated on out-pass
                # progress so they don't steal HBM from the xT stream) ----
                def slab_load(mb, marker=None):
                    t = adjp.tile([P, NB * P], F8, tag="slab",
                                  name=f"slab{mb}")
                    if marker is not None:
                        nc.scalar.copy(t[0:1, 0:1], marker)
                    nc.gpsimd.dma_start(t[:], adjt_d.ap()[mb])
                    return t

                slabs = [slab_load(0, ucols[0:1, 8, 0:1]),
                         slab_load(1, ucols[0:1, 24, 0:1]),
                         slab_load(2, ucols[0:1, 40, 0:1])]
                slabs += [slab_load(mb) for mb in range(3, MB)]

                # ---- stage B: per row-block accumulate + inline epilogue ----
                for mb in range(MB):
                    at = slabs[mb]
                    ya = ps_y.tile([P, FOUT + 1], F32, tag="ya", name="ya")
                    yb = ps_y.tile([P, FOUT + 1], F32, tag="yb", name="yb")
                    for jc in range(NB):
                        dst = ya if jc < 32 else yb
                        nc.tensor.matmul(dst[:], at[:, jc * P:(jc + 1) * P], fo8[jc][:],
                                         start=(jc % 32 == 0),
                                         stop=(jc % 32 == 31))
                    za = comb.tile([P, FOUT + 1], F32, tag="za", name="za")
                    nc.scalar.activation(za[:], ya[:],
                                         mybir.ActivationFunctionType.Copy,
                                         scale=al1[:, mb:mb + 1])
                    t2 = comb.tile([P, FOUT + 1], F32, tag="t2", name="t2")
                    nc.vector.tensor_scalar_mul(t2[:], yb[:],
                                                al2[:, mb:mb + 1])
                    z2 = comb.tile([P, FOUT + 1], F32, tag="z2", name="z2")
                    nc.vector.tensor_add(z2[:], za[:], t2[:])
                    rec = comb.tile([P, 1], F32, tag="rec", name="rec")
                    nc.vector.reciprocal(rec[:], z2[:, FOUT:FOUT + 1])
                    res = comb.tile([P, FOUT], F32, tag="res", name="res")
                    resg = comb.tile([P, FOUT], F32, tag="resg", name="resg")
                    for hh in range(2):
                        cs = slice(hh * 128, hh * 128 + 128)
                        nc.vector.tensor_scalar_mul(res[:, cs], z2[:, cs],
                                                    rec[:])
                        nc.scalar.activation(
                            resg[:, cs], res[:, cs],
                            mybir.ActivationFunctionType.Sigmoid)
                        nc.sync.dma_start(
                            y_d.ap()[mb * P:(mb + 1) * P, cs], resg[:, cs])

    nc.compile()
    return nc


_NC_CACHE = None


def _get_program():
    global _NC_CACHE
    if _NC_CACHE is None:
        _NC_CACHE = build_program()
    return _NC_CACHE


def make_in_maps(x, weight, att_vec, adj):
    import ml_dtypes
    f8 = ml_dtypes.float8_e4m3
    x = np.asarray(x, dtype=np.float32)
    weight = np.ascontiguousarray(np.asarray(weight, dtype=np.float32))
    att_vec = np.asarray(att_vec, dtype=np.float32)

    # xt[s, p, c*1024+n] = x[s*1024+n, c*128+p]
    x8 = x.astype(f8)
    xt = np.ascontiguousarray(
        x8.reshape(NS, 1024, 4, P).transpose(0, 3, 2, 1)).reshape(NS, P, 4096)

    a1 = att_vec[:FOUT, 0]
    a2 = att_vec[FOUT:, 0]
    att3 = np.stack([a1 + a2, a1, a2], axis=1).astype(np.float32)
    wam = (weight @ att3).astype(np.float32)            # [FIN, 3]
    wam_hi = wam.astype(f8)
    wam_lo = (wam - wam_hi.astype(np.float32)).astype(f8)
    wcat = np.concatenate([weight.astype(f8),
                           wam_hi, wam_lo], axis=1)     # [FIN, 262]
    wrhs = np.ascontiguousarray(
        wcat.reshape(4, P, 262).transpose(1, 0, 2)).reshape(P, 4 * 262)

    geo = np.zeros((4, P, P), np.float32)   # [g, p, q]
    q = np.arange(64)
    geo[0, 2 * q, q] = 1.0          # even -> top
    geo[1, 2 * q + 1, q] = 1.0      # odd  -> top
    geo[2, 2 * q, q + 64] = 1.0     # even -> bot
    geo[3, 2 * q + 1, q + 64] = 1.0  # odd -> bot
    geo = np.ascontiguousarray(geo.transpose(1, 0, 2)).reshape(P, 4 * P)

    adj8 = np.asarray(adj, dtype=np.int8).astype(f8)
    in_maps = []
    for c in range(NCORES):
        g = 1.0 if c < 4 else 0.0
        gcol = np.empty((P, 4), np.float32)
        gcol[:, 0] = g * SC
        gcol[:, 1] = (1.0 - g) * SC
        gcol[:, 2] = 0.0
        gcol[:, 3] = 1.0 - g
        selg = np.zeros((P, MB, 32), np.float32)
        for bp in range(MB):
            selg[:, bp, (c * MB + bp) % 32] = g
        # adjt[mb, p, jc*128+i] = adj[c*1024 + mb*128 + i, jc*128 + p]
        A = adj8[c * RPC:(c + 1) * RPC, :]
        adjt = np.ascontiguousarray(
            A.reshape(MB, P, NB, P).transpose(0, 3, 2, 1)).reshape(
                MB, P, NB * P)
        in_maps.append({
            "xt": xt,
            "wrhs": wrhs,
            "geo": geo,
            "adjt": adjt,
            "gcol": gcol,
            "selg": selg,
        })
    return in_maps


def kernel(x, weight, att_vec, adj, _trace=False, _trace_kwargs=None):
    nc = _get_program()
    in_maps = make_in_maps(x, weight, att_vec, adj)
    r = run_bass_kernel_spmd(nc, in_maps, core_ids=list(range(NCORES)),
                             trace=_trace, **(_trace_kwargs or {}))
    y = np.concatenate([r.results[c]["y"] for c in range(NCORES)], axis=0)
    kernel.last_results = r
    return y.astype(np.float32)


# revision 17
# speedup vs baseline: 1.2206x; 1.0304x over previous
"""GAT layer (nn_GAT_layer_67619965108552) as a Trainium2 Bass/Tile SPMD kernel.

Same collapsed math as the previous version (see prep_check.py for the numpy
emulation), restructured for speed:

  * The separate U-pass is gone: the score vectors ride as 6 extra rhs columns
    ([wam_hi | wam_lo] fp8 hi/lo split for precision) on the out-pass matmuls,
    so one pass over xT produces out AND [u, s1, s2] per node.
  * Even/odd pair extraction (tt = s1[even]+s2[odd], u pairs for the alphas)
    happens on the PE with a single resident 0/1 stationary E_eo: one N=24
    matmul per 8 blocks into one PSUM bank, whose columns line up so every
    downstream consumer is a uniform stride-6 2D slice. No DRAM roundtrip.
  * All DRAM operands are host-pre-tiled so every DMA descriptor is a 4-8 KB
    contiguous per-partition run (the old layouts were descriptor-rate bound
    at ~1 KB/descriptor).
  * Stage B runs in mb-slab order: adj arrives as 8 slabs of [all j, 128 i],
    each slab's 64 matmuls accumulate Y1/Y2 for one output row-block, and the
    per-block epilogue (alpha combine, normalize, sigmoid, store) overlaps the
    next slab's matmuls. No epilogue tail.
  * Stage B is all-fp8 (adj 0/1 exact; fo = f*out pre-scaled by 1/4 so the
    normalizer ratio is unchanged and values stay far below the TRN fp8e4
    240 cap). Numpy-emulated end-to-end rel err: 2.5e-3.

Sharding: rows of adj (and the output) across 8 cores, 1024 rows each;
x/weight replicated; every core computes the full out = x@W.
"""
import numpy as np

import concourse.bass as bass
import concourse.tile as tile
from concourse import bacc, mybir
from concourse.bass_utils import run_bass_kernel_spmd

F32 = mybir.dt.float32
F32R = mybir.dt.float32r
BF16 = mybir.dt.bfloat16
F8 = mybir.dt.float8e4

N = 8192
FIN = 512
FOUT = 256
P = 128
NB = N // P        # 64 node blocks
NCORES = 8
RPC = N // NCORES  # 1024 output rows per core
MB = RPC // P      # 8 output row blocks per core
NS = 8             # xT slices
SC = 0.25          # fo pre-scale (cancels in the normalizer ratio)


def build_program():
    nc = bacc.Bacc("TRN2", target_bir_lowering=False, debug=False,
                   num_devices=NCORES)

    xt_d = nc.dram_tensor("xt", [NS, P, 4 * 1024], F8, kind="ExternalInput")
    wrhs_d = nc.dram_tensor("wrhs", [P, 4 * 262], F8, kind="ExternalInput")
    geo_d = nc.dram_tensor("geo", [P, 4 * P], F32R, kind="ExternalInput")
    adjt_d = nc.dram_tensor("adjt", [MB, P, NB * P], F8, kind="ExternalInput")
    # gcol cols: [g*SC, (1-g)*SC, 0, 1-g]
    g_d = nc.dram_tensor("gcol", [P, 4], F32, kind="ExternalInput")
    # selg[p, bp, B] = g * (B == 8c + bp)
    selg_d = nc.dram_tensor("selg", [P, MB, 32], F32, kind="ExternalInput")
    y_d = nc.dram_tensor("y", [RPC, FOUT], F32, kind="ExternalOutput")

    with tile.TileContext(nc) as tc:
        with tc.tile_pool(name="const", bufs=1) as constp, \
             tc.tile_pool(name="adjp", bufs=3) as adjp, \
             tc.tile_pool(name="sa", bufs=3) as sa:

            # ---- constants (scalar HWDGE ring; sync ring streams xT) ----
            wrhs = constp.tile([P, 4, 262], F8)
            nc.scalar.dma_start(wrhs[:], wrhs_d.ap().rearrange(
                "p (c v) -> p c v", v=262))
            geo = constp.tile([P, 4, P], F32R)
            nc.scalar.dma_start(geo[:], geo_d.ap().rearrange(
                "p (g q) -> p g q", q=P))
            gcol = constp.tile([P, 4], F32)
            nc.scalar.dma_start(gcol[:], g_d.ap())
            selg = constp.tile([P, MB, 32], F32)
            nc.scalar.dma_start(selg[:], selg_d.ap())

            dumm = constp.tile([1, 2], F32)
            nc.gpsimd.memset(dumm[:], 0.0)

            # persistent staging: out blocks (col 256 preset to 1.0 so the
            # f-scale pass emits the normalizer column for free)
            outb = [constp.tile([P, FOUT + 1], BF16, name=f"outb{b}")
                    for b in range(NB)]
            for b in range(NB):
                nc.gpsimd.memset(outb[b][:, FOUT:FOUT + 1], 1.0)
            fo8 = [constp.tile([P, FOUT + 1], F8, name=f"fo8_{b}")
                   for b in range(NB)]
            uc6 = constp.tile([P, NB, 6], F32)
            ucols = constp.tile([P, NB, 3], F32R)
            fpm = constp.tile([P, 32], F32)
            al1 = constp.tile([P, MB], F32)
            al2 = constp.tile([P, MB], F32)

            # ---- fused out+score pass over the xT stream ----
            with tc.tile_pool(name="xtp", bufs=NS) as xtp, \
                 tc.tile_pool(name="ps_o", bufs=4, space="PSUM") as ps_o, \
                 tc.tile_pool(name="ps_e", bufs=1, space="PSUM") as ps_e:

                # eoX cols: [tts 32 | ae 32 | be 32], partition-aligned
                eoX = ps_e.tile([P, 96], F32, name="eoX")

                def extract_half(h):
                    # rhs: even/odd blocks of half h; dst cols 16h:16h+16
                    ev = ucols[:, 32 * h:32 * h + 32:2, :]
                    od = ucols[:, 32 * h + 1:32 * h + 32:2, :]
                    for base, pairs in ((0, ((0, ev, 1), (1, ev, 2),
                                             (2, od, 1), (3, od, 2))),
                                        (32, ((0, ev, 0), (2, od, 0))),
                                        (64, ((1, ev, 0), (3, od, 0)))):
                        dst = eoX[:, base + 16 * h:base + 16 * h + 16]
                        for k, (g, rr, v) in enumerate(pairs):
                            nc.tensor.matmul(dst, geo[:, g, :], rr[:, :, v],
                                             start=(k == 0),
                                             stop=(k == len(pairs) - 1))

                def fpm_half(h):
                    hs = slice(16 * h, 16 * h + 16)
                    lrt = sa.tile([P, 16], F32, tag="lrt", name="lrt")
                    nc.vector.tensor_scalar_mul(lrt[:], eoX[:, hs], 0.01)
                    nc.vector.tensor_max(lrt[:], eoX[:, hs], lrt[:])
                    ext = sa.tile([P, 16], F32, tag="ext", name="ext")
                    nc.scalar.activation(ext[:], lrt[:],
                                         mybir.ActivationFunctionType.Exp)
                    nc.vector.tensor_scalar(fpm[:, hs], ext[:], gcol[:, 1:2],
                                            gcol[:, 0:1],
                                            op0=mybir.AluOpType.mult,
                                            op1=mybir.AluOpType.add)

                def scale_one(jc):
                    kc = jc % 32
                    if jc % 2 == 0:
                        nc.scalar.activation(
                            fo8[jc][:], outb[jc][:],
                            mybir.ActivationFunctionType.Copy,
                            scale=fpm[:, kc:kc + 1])
                    else:
                        nc.vector.tensor_scalar_mul(fo8[jc][:], outb[jc][:],
                                                    fpm[:, kc:kc + 1])

                def half_jcs(h):
                    return list(range(16 * h, 16 * h + 16)) + \
                        list(range(32 + 16 * h, 32 + 16 * h + 16))

                pend_scale = []
                for s in range(NS):
                    if s == 5:
                        pend_scale = [jc for jc in half_jcs(0)
                                      if not 40 <= jc <= 47]
                    xs = xtp.tile([P, 4096], F8, tag="xts", name=f"xt{s}")
                    nc.sync.dma_start(xs[:], xt_d.ap()[s])
                    for b8 in range(8):
                        b = s * 8 + b8
                        po = ps_o.tile([P, 262], F32, tag="po", name="po")
                        for c in range(4):
                            nc.tensor.matmul(
                                po[:],
                                xs[:, c * 1024 + b8 * P:
                                   c * 1024 + (b8 + 1) * P],
                                wrhs[:, c, :], start=(c == 0), stop=(c == 3))
                        if b == 1:
                            # exp-table warm, anchored mid-stream
                            nc.scalar.activation(
                                dumm[:, 0:1], ucols[0:1, 0, 0:1],
                                mybir.ActivationFunctionType.Exp)
                        if b == 34:
                            # first-half extraction rides mid-stream (deps
                            # on slices 0-3 are settled by now)
                            extract_half(0)
                            fpm_half(0)
                        if b >= 36 and pend_scale:
                            # drip fo8 scales through the copy stream so the
                            # FIFO ACT/DVE queues never stall the PE
                            scale_one(pend_scale.pop(0))
                        if 40 <= b <= 47:
                            # kc = b-32 is in fpm half 0 (ready by now):
                            # fuse the f-scale into the PSUM drain, skip
                            # outb staging and the separate scale op
                            kc = b - 32
                            if b % 2 == 0:
                                nc.scalar.activation(
                                    fo8[b][:, 0:FOUT], po[:, 0:FOUT],
                                    mybir.ActivationFunctionType.Copy,
                                    scale=fpm[:, kc:kc + 1])
                                nc.vector.tensor_copy(uc6[:, b, :],
                                                      po[:, 256:262])
                                nc.vector.tensor_copy(fo8[b][:, FOUT:],
                                                      fpm[:, kc:kc + 1])
                            else:
                                nc.vector.tensor_scalar_mul(
                                    fo8[b][:, 0:FOUT], po[:, 0:FOUT],
                                    fpm[:, kc:kc + 1])
                                nc.scalar.copy(uc6[:, b, :], po[:, 256:262])
                                nc.scalar.copy(fo8[b][:, FOUT:],
                                               fpm[:, kc:kc + 1])
                        elif b % 2 == 0:
                            nc.scalar.copy(outb[b][:, 0:FOUT], po[:, 0:FOUT])
                            nc.vector.tensor_copy(uc6[:, b, :],
                                                  po[:, 256:262])
                        else:
                            nc.vector.tensor_copy(outb[b][:, 0:FOUT],
                                                  po[:, 0:FOUT])
                            nc.scalar.copy(uc6[:, b, :], po[:, 256:262])
                    sl = slice(s * 8, (s + 1) * 8)
                    nc.vector.tensor_add(ucols[:, sl, :], uc6[:, sl, 0:3],
                                         uc6[:, sl, 3:6])
                for jc in pend_scale:
                    scale_one(jc)
                extract_half(1)
                fpm_half(1)
                for jc in half_jcs(1):
                    scale_one(jc)
                # sigmoid-table warm, anchored on fpm
                nc.scalar.activation(dumm[:, 1:2], fpm[0:1, 0:1],
                                     mybir.ActivationFunctionType.Sigmoid)

                # alphas: aev/bev = exp(lrelu(ae/be)), al = selg-reduce + 1-g
                lra = sa.tile([P, 64], F32, tag="lra", name="lra")
                aeb = sa.tile([P, 64], F32, tag="aeb", name="aeb")
                nc.vector.tensor_copy(aeb[:, 0:32], eoX[:, 32:64])
                nc.vector.tensor_copy(aeb[:, 32:64], eoX[:, 64:96])
                nc.vector.tensor_scalar_mul(lra[:], aeb[:], 0.01)
                nc.vector.tensor_max(lra[:], aeb[:], lra[:])
                nc.scalar.activation(aeb[:], lra[:],
                                     mybir.ActivationFunctionType.Exp)
                for bp in range(MB):
                    m1 = sa.tile([P, 32], F32, tag="alm", name="alm1")
                    nc.vector.tensor_mul(m1[:], aeb[:, 0:32], selg[:, bp, :])
                    nc.vector.tensor_reduce(al1[:, bp:bp + 1], m1[:],
                                            axis=mybir.AxisListType.X,
                                            op=mybir.AluOpType.add)
                    m2 = sa.tile([P, 32], F32, tag="alm", name="alm2")
                    nc.vector.tensor_mul(m2[:], aeb[:, 32:64], selg[:, bp, :])
                    nc.vector.tensor_reduce(al2[:, bp:bp + 1], m2[:],
                                            axis=mybir.AxisListType.X,
                                            op=mybir.AluOpType.add)
                nc.vector.tensor_scalar_add(al1[:], al1[:], gcol[:, 3:4])
                nc.vector.tensor_scalar_add(al2[:], al2[:], gcol[:, 3:4])

            # ---- f-scale pass (jc order = stage-B consumption order) ----
            with tc.tile_pool(name="ps_y", bufs=4, space="PSUM") as ps_y, \
                 tc.tile_pool(name="comb", bufs=3) as comb:

                # ---- adj slabs (SWDGE; early ones gated on out-pass
                # progress so they don't steal HBM from the xT stream) ----
                def slab_load(mb, marker=None):
                    t = adjp.tile([P, NB * P], F8, tag="slab",
                                  name=f"slab{mb}")
                    if marker is not None:
                        nc.scalar.copy(t[0:1, 0:1], marker)
                    nc.gpsimd.dma_start(t[:], adjt_d.ap()[mb])
                    return t

                slabs = [slab_load(0, ucols[0:1, 8, 0:1]),
                         slab_load(1, ucols[0:1, 24, 0:1]),
                         slab_load(2, ucols[0:1, 40, 0:1])]
                slabs += [slab_load(mb) for mb in range(3, MB)]

                # ---- stage B: per row-block accumulate + inline epilogue ----
                for mb in range(MB):
                    at = slabs[mb]
                    ya = ps_y.tile([P, FOUT + 1], F32, tag="ya", name="ya")
                    yb = ps_y.tile([P, FOUT + 1], F32, tag="yb", name="yb")
                    for jc in range(NB):
                        dst = ya if jc < 32 else yb
                        nc.tensor.matmul(dst[:], at[:, jc * P:(jc + 1) * P], fo8[jc][:],
                                         start=(jc % 32 == 0),
                                         stop=(jc % 32 == 31))
                    za = comb.tile([P, FOUT + 1], F32, tag="za", name="za")
                    nc.scalar.activation(za[:], ya[:],
                                         mybir.ActivationFunctionType.Copy,
                                         scale=al1[:, mb:mb + 1])
                    t2 = comb.tile([P, FOUT + 1], F32, tag="t2", name="t2")
                    nc.vector.tensor_scalar_mul(t2[:], yb[:],
                                                al2[:, mb:mb + 1])
                    z2 = comb.tile([P, FOUT + 1], F32, tag="z2", name="z2")
                    nc.vector.tensor_add(z2[:], za[:], t2[:])
                    rec = comb.tile([P, 1], F32, tag="rec", name="rec")
                    nc.vector.reciprocal(rec[:], z2[:, FOUT:FOUT + 1])
                    res = comb.tile([P, FOUT], F32, tag="res", name="res")
                    nc.vector.tensor_scalar_mul(res[:], z2[:, :FOUT], rec[:])
                    resg = comb.tile([P, FOUT], F32, tag="resg", name="resg")
                    nc.scalar.activation(resg[:], res[:],
                                         mybir.ActivationFunctionType.Sigmoid)
                    nc.sync.dma_start(y_d.ap()[mb * P:(mb + 1) * P, :],
                                      resg[:])

    nc.compile()
    return nc


_NC_CACHE = None


def _get_program():
    global _NC_CACHE
    if _NC_CACHE is None:
        _NC_CACHE = build_program()
    return _NC_CACHE


def make_in_maps(x, weight, att_vec, adj):
    import ml_dtypes
    f8 = ml_dtypes.float8_e4m3
    x = np.asarray(x, dtype=np.float32)
    weight = np.ascontiguousarray(np.asarray(weight, dtype=np.float32))
    att_vec = np.asarray(att_vec, dtype=np.float32)

    # xt[s, p, c*1024+n] = x[s*1024+n, c*128+p]
    x8 = x.astype(f8)
    xt = np.ascontiguousarray(
        x8.reshape(NS, 1024, 4, P).transpose(0, 3, 2, 1)).reshape(NS, P, 4096)

    a1 = att_vec[:FOUT, 0]
    a2 = att_vec[FOUT:, 0]
    att3 = np.stack([a1 + a2, a1, a2], axis=1).astype(np.float32)
    wam = (weight @ att3).astype(np.float32)            # [FIN, 3]
    wam_hi = wam.astype(f8)
    wam_lo = (wam - wam_hi.astype(np.float32)).astype(f8)
    wcat = np.concatenate([weight.astype(f8),
                           wam_hi, wam_lo], axis=1)     # [FIN, 262]
    wrhs = np.ascontiguousarray(
        wcat.reshape(4, P, 262).transpose(1, 0, 2)).reshape(P, 4 * 262)

    geo = np.zeros((4, P, P), np.float32)   # [g, p, q]
    q = np.arange(64)
    geo[0, 2 * q, q] = 1.0          # even -> top
    geo[1, 2 * q + 1, q] = 1.0      # odd  -> top
    geo[2, 2 * q, q + 64] = 1.0     # even -> bot
    geo[3, 2 * q + 1, q + 64] = 1.0  # odd -> bot
    geo = np.ascontiguousarray(geo.transpose(1, 0, 2)).reshape(P, 4 * P)

    adj8 = np.asarray(adj, dtype=np.int8).astype(f8)
    in_maps = []
    for c in range(NCORES):
        g = 1.0 if c < 4 else 0.0
        gcol = np.empty((P, 4), np.float32)
        gcol[:, 0] = g * SC
        gcol[:, 1] = (1.0 - g) * SC
        gcol[:, 2] = 0.0
        gcol[:, 3] = 1.0 - g
        selg = np.zeros((P, MB, 32), np.float32)
        for bp in range(MB):
            selg[:, bp, (c * MB + bp) % 32] = g
        # adjt[mb, p, jc*128+i] = adj[c*1024 + mb*128 + i, jc*128 + p]
        A = adj8[c * RPC:(c + 1) * RPC, :]
        adjt = np.ascontiguousarray(
            A.reshape(MB, P, NB, P).transpose(0, 3, 2, 1)).reshape(
                MB, P, NB * P)
        in_maps.append({
            "xt": xt,
            "wrhs": wrhs,
            "geo": geo,
            "adjt": adjt,
            "gcol": gcol,
            "selg": selg,
        })
    return in_maps


def kernel(x, weight, att_vec, adj, _trace=False, _trace_kwargs=None):
    nc = _get_program()
    in_maps = make_in_maps(x, weight, att_vec, adj)
    r = run_bass_kernel_spmd(nc, in_maps, core_ids=list(range(NCORES)),
                             trace=_trace, **(_trace_kwargs or {}))
    y = np.concatenate([r.results[c]["y"] for c in range(NCORES)], axis=0)
    kernel.last_results = r
    return y.astype(np.float32)


# revision 21
# speedup vs baseline: 1.2244x; 1.0031x over previous
"""GAT layer (nn_GAT_layer_67619965108552) as a Trainium2 Bass/Tile SPMD kernel.

Same collapsed math as the previous version (see prep_check.py for the numpy
emulation), restructured for speed:

  * The separate U-pass is gone: the score vectors ride as 6 extra rhs columns
    ([wam_hi | wam_lo] fp8 hi/lo split for precision) on the out-pass matmuls,
    so one pass over xT produces out AND [u, s1, s2] per node.
  * Even/odd pair extraction (tt = s1[even]+s2[odd], u pairs for the alphas)
    happens on the PE with a single resident 0/1 stationary E_eo: one N=24
    matmul per 8 blocks into one PSUM bank, whose columns line up so every
    downstream consumer is a uniform stride-6 2D slice. No DRAM roundtrip.
  * All DRAM operands are host-pre-tiled so every DMA descriptor is a 4-8 KB
    contiguous per-partition run (the old layouts were descriptor-rate bound
    at ~1 KB/descriptor).
  * Stage B runs in mb-slab order: adj arrives as 8 slabs of [all j, 128 i],
    each slab's 64 matmuls accumulate Y1/Y2 for one output row-block, and the
    per-block epilogue (alpha combine, normalize, sigmoid, store) overlaps the
    next slab's matmuls. No epilogue tail.
  * Stage B is all-fp8 (adj 0/1 exact; fo = f*out pre-scaled by 1/4 so the
    normalizer ratio is unchanged and values stay far below the TRN fp8e4
    240 cap). Numpy-emulated end-to-end rel err: 2.5e-3.

Sharding: rows of adj (and the output) across 8 cores, 1024 rows each;
x/weight replicated; every core computes the full out = x@W.
"""
import numpy as np

import concourse.bass as bass
import concourse.tile as tile
from concourse import bacc, mybir
from concourse.bass_utils import run_bass_kernel_spmd

F32 = mybir.dt.float32
F32R = mybir.dt.float32r
BF16 = mybir.dt.bfloat16
F8 = mybir.dt.float8e4

N = 8192
FIN = 512
FOUT = 256
P = 128
NB = N // P        # 64 node blocks
NCORES = 8
RPC = N // NCORES  # 1024 output rows per core
MB = RPC // P      # 8 output row blocks per core
NS = 8             # xT slices
SC = 0.25          # fo pre-scale (cancels in the normalizer ratio)


def build_program():
    nc = bacc.Bacc("TRN2", target_bir_lowering=False, debug=False,
                   num_devices=NCORES)

    xt_d = nc.dram_tensor("xt", [NS, P, 4 * 1024], F8, kind="ExternalInput")
    wrhs_d = nc.dram_tensor("wrhs", [P, 4 * 262], F8, kind="ExternalInput")
    geo_d = nc.dram_tensor("geo", [P, 4 * P], F32R, kind="ExternalInput")
    adjt_d = nc.dram_tensor("adjt", [MB, P, NB * P], F8, kind="ExternalInput")
    # gcol cols: [g*SC, (1-g)*SC, 0, 1-g]
    g_d = nc.dram_tensor("gcol", [P, 4], F32, kind="ExternalInput")
    # selg[p, bp, B] = g * (B == 8c + bp)
    selg_d = nc.dram_tensor("selg", [P, MB, 32], F32, kind="ExternalInput")
    y_d = nc.dram_tensor("y", [RPC, FOUT], F32, kind="ExternalOutput")

    with tile.TileContext(nc) as tc:
        with tc.tile_pool(name="const", bufs=1) as constp, \
             tc.tile_pool(name="adjp", bufs=3) as adjp, \
             tc.tile_pool(name="sa", bufs=3) as sa:

            # ---- constants (scalar HWDGE ring; sync ring streams xT) ----
            wrhs = constp.tile([P, 4, 262], F8)
            nc.scalar.dma_start(wrhs[:], wrhs_d.ap().rearrange(
                "p (c v) -> p c v", v=262))
            geo = constp.tile([P, 4, P], F32R)
            nc.scalar.dma_start(geo[:], geo_d.ap().rearrange(
                "p (g q) -> p g q", q=P))
            gcol = constp.tile([P, 4], F32)
            nc.scalar.dma_start(gcol[:], g_d.ap())
            selg = constp.tile([P, MB, 32], F32)
            nc.scalar.dma_start(selg[:], selg_d.ap())

            dumm = constp.tile([1, 2], F32)
            nc.gpsimd.memset(dumm[:], 0.0)

            # persistent staging: out blocks (col 256 preset to 1.0 so the
            # f-scale pass emits the normalizer column for free)
            outb = [constp.tile([P, FOUT + 1], BF16, name=f"outb{b}")
                    for b in range(NB)]
            for b in range(NB):
                nc.gpsimd.memset(outb[b][:, FOUT:FOUT + 1], 1.0)
            fo8 = [constp.tile([P, FOUT + 1], F8, name=f"fo8_{b}")
                   for b in range(NB)]
            uc6 = constp.tile([P, NB, 6], F32R)
            fpm = constp.tile([P, 32], F32)
            al1 = constp.tile([P, MB], F32)
            al2 = constp.tile([P, MB], F32)

            # ---- fused out+score pass over the xT stream ----
            with tc.tile_pool(name="xtp", bufs=NS) as xtp, \
                 tc.tile_pool(name="ps_o", bufs=4, space="PSUM") as ps_o, \
                 tc.tile_pool(name="ps_e", bufs=1, space="PSUM") as ps_e:

                # eoX cols: [tts 32 | ae 32 | be 32], partition-aligned
                eoX = ps_e.tile([P, 96], F32, name="eoX")

                def extract_half(h):
                    # rhs: even/odd blocks of half h; cols v (wam_hi) and
                    # v+3 (wam_lo) accumulate in PSUM, so the hi+lo add
                    # costs nothing on DVE
                    ev = uc6[:, 32 * h:32 * h + 32:2, :]
                    od = uc6[:, 32 * h + 1:32 * h + 32:2, :]
                    jobs = []     # (region_base, g, rhs, col)
                    for base, pairs in ((0, ((0, ev, 1), (1, ev, 2),
                                             (2, od, 1), (3, od, 2))),
                                        (32, ((0, ev, 0), (2, od, 0))),
                                        (64, ((1, ev, 0), (3, od, 0)))):
                        for g, rr, v in pairs:
                            jobs.append((base, g, rr, v))
                            jobs.append((base, g, rr, v + 3))
                    jobs.sort(key=lambda j: j[1])   # stationary-major
                    left = {0: 8, 32: 4, 64: 4}
                    seen = {0: 0, 32: 0, 64: 0}
                    for base, g, rr, v in jobs:
                        dst = eoX[:, base + 16 * h:base + 16 * h + 16]
                        seen[base] += 1
                        nc.tensor.matmul(dst, geo[:, g, :], rr[:, :, v],
                                         start=(seen[base] == 1),
                                         stop=(seen[base] == left[base]))

                def fpm_half(h):
                    hs = slice(16 * h, 16 * h + 16)
                    lrt = sa.tile([P, 16], F32, tag="lrt", name="lrt")
                    nc.vector.tensor_scalar_mul(lrt[:], eoX[:, hs], 0.01)
                    nc.vector.tensor_max(lrt[:], eoX[:, hs], lrt[:])
                    ext = sa.tile([P, 16], F32, tag="ext", name="ext")
                    nc.scalar.activation(ext[:], lrt[:],
                                         mybir.ActivationFunctionType.Exp)
                    nc.vector.tensor_scalar(fpm[:, hs], ext[:], gcol[:, 1:2],
                                            gcol[:, 0:1],
                                            op0=mybir.AluOpType.mult,
                                            op1=mybir.AluOpType.add)

                def scale_one(jc):
                    kc = jc % 32
                    if jc % 2 == 0:
                        nc.scalar.activation(
                            fo8[jc][:], outb[jc][:],
                            mybir.ActivationFunctionType.Copy,
                            scale=fpm[:, kc:kc + 1])
                    else:
                        nc.vector.tensor_scalar_mul(fo8[jc][:], outb[jc][:],
                                                    fpm[:, kc:kc + 1])

                def half_jcs(h):
                    return list(range(16 * h, 16 * h + 16)) + \
                        list(range(32 + 16 * h, 32 + 16 * h + 16))

                pend_scale = []
                for s in range(NS):
                    if s == 5:
                        pend_scale = [jc for jc in half_jcs(0)
                                      if not 40 <= jc <= 47]
                    xs = xtp.tile([P, 4096], F8, tag="xts", name=f"xt{s}")
                    nc.sync.dma_start(xs[:], xt_d.ap()[s])
                    for b8 in range(8):
                        b = s * 8 + b8
                        po = ps_o.tile([P, 262], F32, tag="po", name="po")
                        for c in range(4):
                            nc.tensor.matmul(
                                po[:],
                                xs[:, c * 1024 + b8 * P:
                                   c * 1024 + (b8 + 1) * P],
                                wrhs[:, c, :], start=(c == 0), stop=(c == 3))
                        if b == 1:
                            # exp-table warm, anchored mid-stream
                            nc.scalar.activation(
                                dumm[:, 0:1], uc6[0:1, 0, 0:1],
                                mybir.ActivationFunctionType.Exp)
                        if b == 34:
                            # first-half extraction rides mid-stream (deps
                            # on slices 0-3 are settled by now)
                            extract_half(0)
                            fpm_half(0)
                        if b >= 36 and pend_scale:
                            # drip fo8 scales through the copy stream so the
                            # FIFO ACT/DVE queues never stall the PE
                            scale_one(pend_scale.pop(0))
                        if 40 <= b <= 47:
                            # kc = b-32 is in fpm half 0 (ready by now):
                            # fuse the f-scale into the PSUM drain, skip
                            # outb staging and the separate scale op
                            kc = b - 32
                            if b % 2 == 0:
                                nc.scalar.activation(
                                    fo8[b][:, 0:FOUT], po[:, 0:FOUT],
                                    mybir.ActivationFunctionType.Copy,
                                    scale=fpm[:, kc:kc + 1])
                                nc.vector.tensor_copy(uc6[:, b, :],
                                                      po[:, 256:262])
                                nc.vector.tensor_copy(fo8[b][:, FOUT:],
                                                      fpm[:, kc:kc + 1])
                            else:
                                nc.vector.tensor_scalar_mul(
                                    fo8[b][:, 0:FOUT], po[:, 0:FOUT],
                                    fpm[:, kc:kc + 1])
                                nc.scalar.copy(uc6[:, b, :], po[:, 256:262])
                                nc.scalar.copy(fo8[b][:, FOUT:],
                                               fpm[:, kc:kc + 1])
                        elif b % 2 == 0:
                            nc.scalar.copy(outb[b][:, 0:FOUT], po[:, 0:FOUT])
                            nc.vector.tensor_copy(uc6[:, b, :],
                                                  po[:, 256:262])
                        else:
                            nc.vector.tensor_copy(outb[b][:, 0:FOUT],
                                                  po[:, 0:FOUT])
                            nc.scalar.copy(uc6[:, b, :], po[:, 256:262])
                for jc in pend_scale:
                    scale_one(jc)
                extract_half(1)
                fpm_half(1)
                for jc in half_jcs(1):
                    scale_one(jc)
                # sigmoid-table warm, anchored on fpm
                nc.scalar.activation(dumm[:, 1:2], fpm[0:1, 0:1],
                                     mybir.ActivationFunctionType.Sigmoid)

                # alphas: aev/bev = exp(lrelu(ae/be)), al = selg-reduce + 1-g
                lra = sa.tile([P, 64], F32, tag="lra", name="lra")
                aeb = sa.tile([P, 64], F32, tag="aeb", name="aeb")
                nc.vector.tensor_copy(aeb[:, 0:32], eoX[:, 32:64])
                nc.vector.tensor_copy(aeb[:, 32:64], eoX[:, 64:96])
                nc.vector.tensor_scalar_mul(lra[:], aeb[:], 0.01)
                nc.vector.tensor_max(lra[:], aeb[:], lra[:])
                nc.scalar.activation(aeb[:], lra[:],
                                     mybir.ActivationFunctionType.Exp)
                for bp in range(MB):
                    m1 = sa.tile([P, 32], F32, tag="alm", name="alm1")
                    nc.vector.tensor_mul(m1[:], aeb[:, 0:32], selg[:, bp, :])
                    nc.vector.tensor_reduce(al1[:, bp:bp + 1], m1[:],
                                            axis=mybir.AxisListType.X,
                                            op=mybir.AluOpType.add)
                    m2 = sa.tile([P, 32], F32, tag="alm", name="alm2")
                    nc.vector.tensor_mul(m2[:], aeb[:, 32:64], selg[:, bp, :])
                    nc.vector.tensor_reduce(al2[:, bp:bp + 1], m2[:],
                                            axis=mybir.AxisListType.X,
                                            op=mybir.AluOpType.add)
                nc.vector.tensor_scalar_add(al1[:], al1[:], gcol[:, 3:4])
                nc.vector.tensor_scalar_add(al2[:], al2[:], gcol[:, 3:4])

            # ---- f-scale pass (jc order = stage-B consumption order) ----
            with tc.tile_pool(name="ps_y", bufs=4, space="PSUM") as ps_y, \
                 tc.tile_pool(name="comb", bufs=3) as comb:

                # ---- adj slabs (SWDGE; early ones gated on out-pass
                # progress so they don't steal HBM from the xT stream) ----
                def slab_load(mb, marker=None):
                    t = adjp.tile([P, NB * P], F8, tag="slab",
                                  name=f"slab{mb}")
                    if marker is not None:
                        nc.scalar.copy(t[0:1, 0:1], marker)
                    nc.gpsimd.dma_start(t[:], adjt_d.ap()[mb])
                    return t

                slabs = [slab_load(0, uc6[0:1, 8, 0:1]),
                         slab_load(1, uc6[0:1, 24, 0:1]),
                         slab_load(2, uc6[0:1, 40, 0:1])]
                slabs += [slab_load(mb) for mb in range(3, MB)]

                # ---- stage B: per row-block accumulate + inline epilogue ----
                for mb in range(MB):
                    at = slabs[mb]
                    ya = ps_y.tile([P, FOUT + 1], F32, tag="ya", name="ya")
                    yb = ps_y.tile([P, FOUT + 1], F32, tag="yb", name="yb")
                    for jc in range(NB):
                        dst = ya if jc < 32 else yb
                        nc.tensor.matmul(dst[:], at[:, jc * P:(jc + 1) * P], fo8[jc][:],
                                         start=(jc % 32 == 0),
                                         stop=(jc % 32 == 31))
                    za = comb.tile([P, FOUT + 1], F32, tag="za", name="za")
                    nc.scalar.activation(za[:], ya[:],
                                         mybir.ActivationFunctionType.Copy,
                                         scale=al1[:, mb:mb + 1])
                    t2 = comb.tile([P, FOUT + 1], F32, tag="t2", name="t2")
                    nc.vector.tensor_scalar_mul(t2[:], yb[:],
                                                al2[:, mb:mb + 1])
                    z2 = comb.tile([P, FOUT + 1], F32, tag="z2", name="z2")
                    nc.vector.tensor_add(z2[:], za[:], t2[:])
                    rec = comb.tile([P, 1], F32, tag="rec", name="rec")
                    nc.vector.reciprocal(rec[:], z2[:, FOUT:FOUT + 1])
                    res = comb.tile([P, FOUT], F32, tag="res", name="res")
                    nc.vector.tensor_scalar_mul(res[:], z2[:, :FOUT], rec[:])
                    resg = comb.tile([P, FOUT], F32, tag="resg", name="resg")
                    nc.scalar.activation(resg[:], res[:],
                                         mybir.ActivationFunctionType.Sigmoid)
                    nc.sync.dma_start(y_d.ap()[mb * P:(mb + 1) * P, :],
                                      resg[:])

    nc.compile()
    return nc


_NC_CACHE = None


def _get_program():
    global _NC_CACHE
    if _NC_CACHE is None:
        _NC_CACHE = build_program()
    return _NC_CACHE


def make_in_maps(x, weight, att_vec, adj):
    import ml_dtypes
    f8 = ml_dtypes.float8_e4m3
    x = np.asarray(x, dtype=np.float32)
    weight = np.ascontiguousarray(np.asarray(weight, dtype=np.float32))
    att_vec = np.asarray(att_vec, dtype=np.float32)

    # xt[s, p, c*1024+n] = x[s*1024+n, c*128+p]
    x8 = x.astype(f8)
    xt = np.ascontiguousarray(
        x8.reshape(NS, 1024, 4, P).transpose(0, 3, 2, 1)).reshape(NS, P, 4096)

    a1 = att_vec[:FOUT, 0]
    a2 = att_vec[FOUT:, 0]
    att3 = np.stack([a1 + a2, a1, a2], axis=1).astype(np.float32)
    wam = (weight @ att3).astype(np.float32)            # [FIN, 3]
    wam_hi = wam.astype(f8)
    wam_lo = (wam - wam_hi.astype(np.float32)).astype(f8)
    wcat = np.concatenate([weight.astype(f8),
                           wam_hi, wam_lo], axis=1)     # [FIN, 262]
    wrhs = np.ascontiguousarray(
        wcat.reshape(4, P, 262).transpose(1, 0, 2)).reshape(P, 4 * 262)

    geo = np.zeros((4, P, P), np.float32)   # [g, p, q]
    q = np.arange(64)
    geo[0, 2 * q, q] = 1.0          # even -> top
    geo[1, 2 * q + 1, q] = 1.0      # odd  -> top
    geo[2, 2 * q, q + 64] = 1.0     # even -> bot
    geo[3, 2 * q + 1, q + 64] = 1.0  # odd -> bot
    geo = np.ascontiguousarray(geo.transpose(1, 0, 2)).reshape(P, 4 * P)

    adj8 = np.asarray(adj, dtype=np.int8).astype(f8)
    in_maps = []
    for c in range(NCORES):
        g = 1.0 if c < 4 else 0.0
        gcol = np.empty((P, 4), np.float32)
        gcol[:, 0] = g * SC
        gcol[:, 1] = (1.0 - g) * SC
        gcol[:, 2] = 0.0
        gcol[:, 3] = 1.0 - g
        selg = np.zeros((P, MB, 32), np.float32)
        for bp in range(MB):
            selg[:, bp, (c * MB + bp) % 32] = g
        # adjt[mb, p, jc*128+i] = adj[c*1024 + mb*128 + i, jc*128 + p]
        A = adj8[c * RPC:(c + 1) * RPC, :]
        adjt = np.ascontiguousarray(
            A.reshape(MB, P, NB, P).transpose(0, 3, 2, 1)).reshape(
                MB, P, NB * P)
        in_maps.append({
            "xt": xt,
            "wrhs": wrhs,
            "geo": geo,
            "adjt": adjt,
            "gcol": gcol,
            "selg": selg,
        })
    return in_maps


def kernel(x, weight, att_vec, adj, _trace=False, _trace_kwargs=None):
    nc = _get_program()
    in_maps = make_in_maps(x, weight, att_vec, adj)
    r = run_bass_kernel_spmd(nc, in_maps, core_ids=list(range(NCORES)),
                             trace=_trace, **(_trace_kwargs or {}))
    y = np.concatenate([r.results[c]["y"] for c in range(NCORES)], axis=0)
    kernel.last_results = r
    return y.astype(np.float32)
